# revision 62
# baseline (speedup 1.0000x reference)
"""Trainium2 Bass kernel for nn_AttentionLayer (B=4, T=2048, C=1024, H=16).

Sharding (8 cores): core c = (batch b = c//2, head-group g = c%2).
Data parallel on batch, tensor parallel on heads: each core computes the
qkv projection for its 8 heads, causal flash-attention, and a partial
output projection (row split of w_proj). Host sums the two partials per
batch and re-transposes.

Per-core kernel (Bass/Tile):
  phase A: qkv projection in f32r (TF32-like).  Q^T/K^T produced in
           [head_dim, t] layout, V in [t, head_dim] layout with an
           appended ones column; all stored bf16 in SBUF.
  phase B: causal attention per head-pair.  S^T = K^T.T @ Q^T, the two
           heads of a pair issued to PE row-groups (0,0)/(64,0) so they
           run concurrently on HW; causal mask applied on PE by
           accumulating a constant -BIG triangle into the diagonal
           128-col band (exp then yields exact zeros); exp on ACT (PSUM
           f32 in, bf16 out); O^T = [V|1].T @ P^T accumulated in PSUM,
           row 64 gives softmax denominators.  Each pair's AV batch and
           normalization are deferred one pair and run as priority
           filler during the next pair's S/exp stream; o-banks are
           copied to SBUF immediately so the norm chain (one batched
           multi-pass reciprocal for both heads + selector-matmul
           broadcast + DVE multiplies) stays off the critical path.
  phase C: out^T = w_p^T.T @ y^T in bf16 + bias (bias only on g=0).

  Emission uses a virtual-clock list scheduler: attention chunks are
  emitted in dependency order, and whenever the PE stream would stall
  on ACT (exp) latency, projection / output-projection matmuls are
  spliced in as filler.  PSUM rings are dedicated (st / o / grp) so
  long-lived accumulators never alias short-lived group tiles.

All DRAM tensors are host-pre-tiled so every DMA is one contiguous block.
"""
from collections import deque
from contextlib import ExitStack

import numpy as np

import concourse.bacc as bacc
import concourse.mybir as mybir
import concourse.tile as tile
from concourse.bass_utils import run_bass_kernel_spmd

F32 = mybir.dt.float32
F32R = mybir.dt.float32r
BF16 = mybir.dt.bfloat16
AF = mybir.ActivationFunctionType

B, T, C, H = 4, 2048, 1024, 16
HD = C // H          # 64
NH = H // 2          # heads per core: 8
QCOLS = NH * HD      # 512

# virtual-clock cost model (calibrated against TimelineSim, which matched
# HW within 1% on the S+exp ablation), ns
SEM = 120.0      # semaphore propagation to a dependent engine
PE_DRAIN = 273.0  # PE pipeline drain (173) + sem before a consumer sees PSUM

def _mm(fd):
    return (6.0 + fd) / 2.4 + 5.0

def _act(fd):
    return (fd + 222.0) / 1.2 + 60.0

def _dve(fd, fixed=151.0, acc=1.0):
    return (fixed + fd / acc) / 0.96 + 70.0


def build(T=T, C=C, NH=NH, HD=HD, TQ=512, loop_iters=1, variant="full"):
    assert C % 128 == 0 and T % TQ == 0 and TQ % 128 == 0
    NP = NH // 2              # head pairs
    CT = C // 128             # contraction tiles
    NTB = T // TQ             # time blocks
    TT = T // 128             # tk tiles
    NO = C // 128             # out row tiles
    QC = NH * HD
    scale = 1.0 / (HD ** 0.5)

    nc = bacc.Bacc()
    xT = nc.declare_dram_parameter("xT", [CT, NTB, 128, TQ], F32R, isOutput=False)
    wqkT = nc.declare_dram_parameter("wqkT", [2, CT, 128, QC], F32R, isOutput=False)
    wvT = nc.declare_dram_parameter("wvT", [CT, 128, QC], F32R, isOutput=False)
    wpT = nc.declare_dram_parameter("wpT", [NP, 128, C], BF16, isOutput=False)
    bias = nc.declare_dram_parameter("bias", [128, NO], F32, isOutput=False)
    outT = nc.declare_dram_parameter("outT", [NO, NTB, 128, TQ], F32, isOutput=True)

    with tile.TileContext(nc) as tc, ExitStack() as ctx:
        # long-lived pools first (stack allocator)
        const_pool = ctx.enter_context(tc.tile_pool(name="const", bufs=1))
        wqk_pool = ctx.enter_context(tc.tile_pool(name="wqk", bufs=2 * CT))
        wv_pool = ctx.enter_context(tc.tile_pool(name="wv", bufs=CT))
        wp_pool = ctx.enter_context(tc.tile_pool(name="wp", bufs=NP))
        qt_pool = ctx.enter_context(tc.tile_pool(name="qt", bufs=NP * NTB))
        kt_pool = ctx.enter_context(tc.tile_pool(name="kt", bufs=NP * NTB))
        yt_pool = ctx.enter_context(tc.tile_pool(name="yt", bufs=NP * NTB))
        v_pool = ctx.enter_context(tc.tile_pool(name="v", bufs=TT))
        xs_pool = ctx.enter_context(tc.tile_pool(name="xs", bufs=2 * CT))
        pt_pool = ctx.enter_context(tc.tile_pool(name="pt", bufs=6))
        rc_pool = ctx.enter_context(tc.tile_pool(name="rc", bufs=3))
        osb_pool = ctx.enter_context(tc.tile_pool(name="osb", bufs=3))

        bias_sb = const_pool.tile([128, NO], F32, tag="bias", name="bias_sb")
        nc.sync.dma_start(bias_sb[:], bias[:])
        ones_sb = const_pool.tile([128, NH], BF16, tag="ones", name="ones_sb")
        nc.gpsimd.memset(ones_sb[:], 1.0)
        # causal band mask: mask_a[x,y]=1 iff y>=x, applied to the single
        # 128-wide diagonal band of each S tile
        bf = mybir.dt.np(BF16)
        ii = np.arange(128)[:, None]
        # negtri[k, q] = -BIG where q < k: accumulated onto diagonal S bands
        # via an identity-stationary matmul, so exp yields exact zeros there.
        negtri_np = np.where(np.arange(128)[None, :] < ii, -30000.0, 0.0).astype(bf)
        eye_np = np.eye(128, dtype=np.float32).astype(bf)
        sel_np = np.zeros((65, 64), np.float32)
        sel_np[0, :] = 1.0
        sel_np[64, :] = 1.0
        negtri_dram = nc.inline_tensor(negtri_np, name="negtri")
        eye_dram = nc.inline_tensor(eye_np, name="eye")
        sel_dram = nc.inline_tensor(sel_np, name="sel")
        negtri = const_pool.tile([128, 128], BF16, tag="negtri", name="negtri_sb")
        eye_sb = const_pool.tile([128, 128], BF16, tag="eye", name="eye_sb")
        sel_sb = const_pool.tile([65, 64], F32R, tag="sel", name="sel_sb")
        nc.gpsimd.dma_start(negtri[:], negtri_dram[:])
        nc.gpsimd.dma_start(eye_sb[:], eye_dram[:])
        nc.gpsimd.dma_start(sel_sb[:], sel_dram[:])

        # persistent weights (loaded once, resident in SBUF)
        wp_sb = [wp_pool.tile([128, C], BF16, tag="wp", name="wp") for _ in range(NP)]
        for p in range(NP):
            nc.sync.dma_start(wp_sb[p][:], wpT[p])
        wqk_sb = [[wqk_pool.tile([128, QC], F32R, tag="wqk", name="wqk")
                   for _ in range(CT)] for _ in range(2)]
        for half in range(2):
            for c in range(CT):
                nc.sync.dma_start(wqk_sb[half][c][:], wqkT[half, c])
        wv_sb = [wv_pool.tile([128, QC], F32R, tag="wv", name="wv") for _ in range(CT)]
        for c in range(CT):
            nc.sync.dma_start(wv_sb[c][:], wvT[c])

        qt = {}
        kt = {}
        vt = []
        yt = {}

        def body():
            qt.clear(); kt.clear(); vt.clear(); yt.clear()
            ctx2 = ExitStack()
            # PSUM: st 2x2 banks + o 2x1 + grp 2x1 = 8 banks
            st_pool = ctx2.enter_context(tc.tile_pool(name="st", bufs=2, space="PSUM"))
            o_pool = ctx2.enter_context(tc.tile_pool(name="o", bufs=2, space="PSUM"))
            grp_pool = ctx2.enter_context(tc.tile_pool(name="grp", bufs=2, space="PSUM"))

            class Sched:
                """Emission-time list scheduler with virtual engine clocks.

                Filler generators yield None after emitting a chunk of work,
                or a time value ("blocked until") when they cannot emit yet.
                `prio` (deferred AV/norm batches) pre-empts `bulk`
                (projection / output projection); FIFO within each class.
                """
                def __init__(self):
                    self.vpe = 0.0
                    self.vact = 0.0
                    self.vdve = 0.0
                    self.prio = deque()
                    self.bulk = deque()
                    self.force = False
                    self._blk = None

                def pe(self, ns):
                    self.vpe += ns

                def dve(self, ns, after=None):
                    t0 = max(self.vdve, self.vpe if after is None else after)
                    self.vdve = t0 + ns
                    return self.vdve

                def wait_until(self, t):
                    while not self.force and self.vpe < t:
                        yield t

                def _step_filler(self):
                    while self.prio:  # strict FIFO within prio
                        g = self.prio[0]
                        try:
                            b = next(g)
                        except StopIteration:
                            self.prio.popleft()
                            continue
                        if b is None:
                            return True
                        if self._blk is None or b < self._blk:
                            self._blk = b
                        break  # prio front blocked: fall back to bulk
                    for _ in range(len(self.bulk)):
                        g = self.bulk[0]
                        try:
                            b = next(g)
                        except StopIteration:
                            self.bulk.popleft()
                            continue
                        if b is None:
                            return True
                        if self._blk is None or b < self._blk:
                            self._blk = b
                        self.bulk.rotate(-1)  # let later bulk gens try
                    return False

                def fill_to(self, t):
                    while self.vpe < t:
                        self._blk = None
                        if not self._step_filler():
                            if self._blk is None:
                                break  # no filler work left at all
                            self.vpe = max(self.vpe, min(self._blk, t))

                def drain(self, g):
                    for dq in (self.prio, self.bulk):
                        try:
                            dq.remove(g)
                        except ValueError:
                            pass
                    self.force = True
                    for _ in g:
                        pass
                    self.force = False

                def drain_all(self):
                    for dq in (self.prio, self.bulk):
                        while dq:
                            self.drain(dq[0])

            sch = Sched()

            def gen_proj(tb):
                """Phase A for block tb; yields after ~2 matmuls of work."""
                xs = [xs_pool.tile([128, TQ], F32R, tag="xs", name="xs")
                      for _ in range(CT)]
                for c in range(CT):
                    nc.sync.dma_start(xs[c][:], xT[c, tb])
                yield
                with nc.allow_low_precision(reason="bf16 attention operands"):
                    def qk_chunk(half, jp):
                        ps = grp_pool.tile([128, TQ], F32, tag="grp", name="mm")
                        for c in range(CT):
                            nc.tensor.matmul(
                                ps[:], wqk_sb[half][c][:, 128 * jp:128 * (jp + 1)],
                                xs[c][:], start=(c == 0), stop=(c == CT - 1))
                            sch.pe(_mm(TQ))
                            yield
                        pool = qt_pool if half == 0 else kt_pool
                        dst = pool.tile([128, TQ], BF16, tag="t", name="qk")
                        nc.vector.tensor_copy(dst[:], ps[:])
                        sch.dve(_dve(TQ, 120.0))
                        (qt if half == 0 else kt)[(jp, tb)] = dst
                        yield

                    def v_chunk(ti):
                        ps = grp_pool.tile([128, QC], F32, tag="grp", name="mmv")
                        for c in range(CT):
                            nc.tensor.matmul(ps[:], xs[c][:, 128 * ti:128 * (ti + 1)],
                                             wv_sb[c][:], start=(c == 0), stop=(c == CT - 1))
                            sch.pe(_mm(QC))
                            yield
                        vtile = v_pool.tile([128, NH * (HD + 1)], BF16, tag="v", name="v")
                        v3 = vtile[:].rearrange("p (h d) -> p h d", d=HD + 1)
                        nc.vector.tensor_copy(v3[:, :, 0:HD],
                                              ps[:].rearrange("p (h d) -> p h d", d=HD))
                        nc.vector.tensor_copy(v3[:, :, HD], ones_sb[:])
                        sch.dve(_dve(QC, 120.0) + _dve(NH, 120.0))
                        assert len(vt) == tb * (TQ // 128) + ti
                        vt.append(vtile)
                        yield

                    # pair-0 q/k and all V first so the block's attention can
                    # start while the remaining pairs project as filler
                    yield from qk_chunk(0, 0)
                    yield from qk_chunk(1, 0)
                    for ti in range(TQ // 128):
                        yield from v_chunk(ti)
                    for jp in range(1, NP):
                        yield from qk_chunk(0, jp)
                        yield from qk_chunk(1, jp)

            def gen_outproj(tb):
                """Phase C for block tb (filler work). Must not emit before
                the norm tails that write this block's yt tiles; blocks 0..2
                are additionally held in reserve as late-attention filler."""
                while (norms_done[tb] < NP
                       or (tb < NTB - 1 and state["cur"] < NTB - 1)):
                    yield 1e18
                for ot in range(NO):
                    ps = grp_pool.tile([128, TQ], F32, tag="grp", name="mmo")
                    for p in range(NP):
                        nc.tensor.matmul(ps[:], wp_sb[p][:, 128 * ot:128 * (ot + 1)],
                                         yt[(p, tb)][:], start=(p == 0), stop=(p == NP - 1))
                        sch.pe(_mm(TQ))
                        yield
                    osb = osb_pool.tile([128, TQ], F32, tag="osb", name="osb")
                    nc.vector.tensor_scalar_add(osb[:], ps[:], bias_sb[:, ot:ot + 1])
                    sch.dve(_dve(TQ))
                    nc.sync.dma_start(outT[ot, tb], osb[:])
                    yield

            # deferred-pair state: previous pair awaiting its norm tail
            state = {"prev": None, "o_free": 0.0, "done": 0, "seq": 0, "cur": 0}
            norms_done = [0] * NTB
            pt_const = None
            if variant == "avnodep":
                pt_const = const_pool.tile([128, 2 * TQ], BF16, tag="ptc",
                                           name="ptc", bufs=1)
                nc.gpsimd.memset(pt_const[:], 0.001)

            def gen_deferred(p, qi, avlist, qtile, ntk, myseq):
                """Norm tail of the previous pair, then this pair's AV batch
                (h0 sweep then h1 sweep) and reciprocals. Runs as priority
                filler during the NEXT pair's S/exp stream."""
                with nc.allow_low_precision(reason="bf16 attention operands"):
                    while state["done"] != myseq - 1:
                        yield 1e18  # wait for the previous deferred unit
                    if state["prev"] is not None:
                        emit_normtail()
                        yield None
                    nonorm = variant in ("avnonorm",)
                    # AV batch: o tiles allocated here (ring order = use order).
                    # h0/h1 interleaved per k-tile so each pt slot frees as
                    # early as possible (the next pair's exp stream reuses it)
                    o0 = o_pool.tile([HD + 1, TQ], F32, tag="o", name="o0")
                    o1 = o_pool.tile([HD + 1, TQ], F32, tag="o", name="o1")
                    yield from sch.wait_until(state["o_free"])
                    for tki, (pt, w, dlt) in enumerate(avlist):
                        vtile = vt[tki]
                        v3 = vtile[:].rearrange("p (h d) -> p h d", d=HD + 1)
                        for h, o in ((0, o0), (1, o1)):
                            nc.tensor.matmul(o[:, dlt:TQ], v3[:, 2 * p + h, :],
                                             pt[:, h * w:(h + 1) * w],
                                             start=(tki == 0), stop=(tki == ntk - 1))
                        sch.pe(2 * _mm(w))
                        yield None
                    if not nonorm:
                        # free the PSUM o-banks immediately: copy to SBUF
                        # (and stage the denominator rows into partitions 0
                        # and 64 of one tile); the norm chain then runs
                        # entirely off the copies
                        oc0 = rc_pool.tile([HD + 1, TQ], F32, tag="oc0", name="oc0", bufs=2)
                        oc1 = rc_pool.tile([HD + 1, TQ], F32, tag="oc1", name="oc1", bufs=2)
                        rcf = rc_pool.tile([65, TQ], F32, tag="rcf", name="rcf", bufs=2)
                        nc.vector.tensor_copy(oc0[:], o0[:])
                        nc.vector.tensor_copy(oc1[:], o1[:])
                        nc.vector.tensor_copy(rcf[0:1, :], o0[HD:HD + 1, :])
                        nc.vector.tensor_copy(rcf[64:65, :], o1[HD:HD + 1, :])
                        ofree = sch.dve(4 * _dve(TQ, 120.0), after=sch.vpe + PE_DRAIN)
                        state["o_free"] = ofree + SEM
                        yield None
                        # one reciprocal covers both heads (cost scales with
                        # FD, not partitions): fast Newton-Raphson approx
                        # (~18 bits, 5x faster than multi-pass InstReciprocal)
                        # + f32r rounding copy for the broadcast matmul
                        rc2f = rc_pool.tile([65, TQ], F32, tag="rc2f", name="rc2f", bufs=2)
                        rc2 = rc_pool.tile([65, TQ], F32R, tag="rc2", name="rc2", bufs=2)
                        nc.vector.reciprocal_approx_fast(rc2f[:], rcf[:])
                        nc.vector.tensor_copy(rc2[:], rc2f[:])
                        rcA = rc2[0:1, :]
                        rcB = rc2[64:65, :]
                        sch.dve(3 * _dve(TQ, 58.0))
                        state["prev"] = (p, qi, oc0, oc1, rcA, rcB)
                    state["done"] = myseq

            def emit_normtail():
                p, qi, o0, o1, rcA, rcB = state["prev"]
                bc0 = grp_pool.tile([HD, TQ], F32, tag="grp", name="bc0")
                bc1 = grp_pool.tile([HD, TQ], F32, tag="grp", name="bc1")
                nc.tensor.matmul(bc0[:], sel_sb[0:1, :], rcA[:], start=True, stop=True)
                nc.tensor.matmul(bc1[:], sel_sb[64:65, :], rcB[:], start=True, stop=True)
                sch.pe(2 * _mm(TQ))
                # muls read bc straight from PSUM (other operand is SBUF)
                ytile = yt[(p, qi)]
                nc.vector.tensor_mul(ytile[0:64, :], o0[0:HD, :], bc0[:])
                nc.vector.tensor_mul(ytile[64:128, :], o1[0:HD, :], bc1[:])
                sch.dve(2 * _dve(TQ, 120.0), after=sch.vpe + PE_DRAIN)
                state["prev"] = None
                norms_done[qi] += 1

            def flush_norm():
                """Emit the final pair's norm tail (after its deferred ran)."""
                while state["prev"] is not None or sch.prio:
                    if sch.prio:
                        sch.drain(sch.prio[0])
                    elif state["prev"] is not None:
                        emit_normtail()

            st_free = [0.0, 0.0]

            def ensure_pair(p, qi, ntk):
                """Pull the minimum projection chunks needed for pair (p, qi)
                to be emittable (tiles must exist at emission time)."""
                def ok():
                    return ((p, qi) in qt and len(vt) >= ntk
                            and all((p, tb) in kt for tb in range(qi + 1)))
                for tb in range(qi + 1):
                    g = proj_gens[tb]
                    while not ok():
                        try:
                            next(g)
                        except StopIteration:
                            break
                assert ok()

            def emit_attention(qi):
                state["cur"] = qi
                tq0 = qi * TQ
                ntk = (tq0 + TQ) // 128
                with nc.allow_low_precision(reason="bf16 attention operands"):
                    for p in range(NP):
                        ensure_pair(p, qi, ntk)
                        qtile = qt[(p, qi)]
                        if variant != "sexp":
                            ytile = yt_pool.tile([128, TQ], BF16, tag="yt", name="y")
                            yt[(p, qi)] = ytile
                        avlist = []
                        for tki in range(ntk):
                            tk0 = tki * 128
                            dlt = max(0, tk0 - tq0)
                            w = TQ - dlt
                            diag = tk0 >= tq0
                            ktile = kt[(p, tk0 // TQ)]
                            koff = tk0 % TQ
                            # st ring slot reuse: wait for exp(i-2) to clear
                            sch.fill_to(st_free[tki % 2])
                            st = st_pool.tile([128, 2 * TQ], F32, tag="st", name="st")
                            nc.tensor.matmul(st[:, 0:w], ktile[0:64, koff:koff + 128],
                                             qtile[0:64, dlt:TQ], start=True, stop=not diag)
                            nc.tensor.matmul(st[:, TQ:TQ + w], ktile[64:128, koff:koff + 128],
                                             qtile[64:128, dlt:TQ], start=True, stop=not diag)
                            sch.pe(_mm(w) + 4.0)  # row-groups (0,0)/(64,0) run concurrently
                            if diag:
                                # mask the 128-wide diagonal band: += -BIG triangle
                                nc.tensor.matmul(st[:, 0:128], eye_sb[:], negtri[:],
                                                 start=False, stop=True)
                                nc.tensor.matmul(st[:, TQ:TQ + 128], eye_sb[:], negtri[:],
                                                 start=False, stop=True)
                                sch.pe(2 * _mm(128))
                            pt = pt_pool.tile([128, 2 * w], BF16, tag="pt", name="pt",
                                              padded_shape=[128, 2 * TQ])
                            st_v = st[:].rearrange("p (h q) -> p h q", q=TQ)[:, :, 0:w]
                            pt_v = pt[:].rearrange("p (h q) -> p h q", h=2)
                            nc.scalar.activation(pt_v, st_v, AF.Exp, scale=scale)
                            exp_end = max(sch.vact, sch.vpe + PE_DRAIN) + _act(2 * w)
                            sch.vact = exp_end
                            st_free[tki % 2] = exp_end + SEM
                            avlist.append((pt_const[:, 0:2 * w] if pt_const is not None
                                           else pt, w, dlt))
                        if variant == "sexp":
                            yt[(p, qi)] = qtile
                            norms_done[qi] += 1
                            continue
                        if variant == "avnonorm":
                            yt[(p, qi)] = qtile
                            norms_done[qi] += 1
                        state["seq"] += 1
                        sch.prio.append(gen_deferred(p, qi, avlist, qtile, ntk,
                                                     state["seq"]))

            # ---- main emission ----
            proj_gens = [gen_proj(tb) for tb in range(NTB)]
            if variant == "projout":
                # ablation: pure PE pipeline (no attention chains)
                for tb in range(NTB):
                    sch.drain(proj_gens[tb])
                    for p in range(NP):
                        yt[(p, tb)] = qt[(p, tb)]
                    norms_done[tb] = NP
                    sch.drain(gen_outproj(tb))
                ctx2.close()
                return
            if variant == "serial":
                for tb in range(NTB):
                    sch.drain(proj_gens[tb])
                for qi in range(NTB):
                    emit_attention(qi)
                flush_norm()
                for qi in range(NTB):
                    sch.drain(gen_outproj(qi))
                ctx2.close()
                return
            sch.bulk.append(proj_gens[0])
            for qi in range(NTB):
                if qi + 1 < NTB:
                    sch.bulk.append(proj_gens[qi + 1])
                emit_attention(qi)
                sch.bulk.append(gen_outproj(qi))
            flush_norm()
            sch.drain_all()
            ctx2.close()

        if loop_iters == 1:
            body()
        else:
            with tc.For_i(0, loop_iters, 1):
                body()
    nc.finalize()
    return nc


def _tile2d(a, pr, pc):
    """[R, S] -> [R//pr, S//pc, pr, pc] contiguous tiles."""
    R, S = a.shape
    return np.ascontiguousarray(
        a.reshape(R // pr, pr, S // pc, pc).transpose(0, 2, 1, 3))


def shard_inputs(x, w_attn, w_proj, b_proj, TQ=512):
    """Returns in_maps for 8 cores: core c = (b=c//2, g=c%2)."""
    CT = C // 128
    NP = NH // 2
    bf = mybir.dt.np(BF16)
    wq, wk, wv = w_attn[0:C], w_attn[C:2 * C], w_attn[2 * C:3 * C]
    x = np.asarray(x)
    in_maps = []
    for core in range(8):
        b = core // 2
        g = core % 2
        rows = slice(g * QCOLS, (g + 1) * QCOLS)
        xTt = _tile2d(np.asarray(x[b]).T, 128, TQ)                 # [CT,NTB,128,TQ]
        wqkT_flat = np.concatenate([wq[rows], wk[rows]], 0).T      # [C, 2QC]
        wqkTt = np.ascontiguousarray(
            wqkT_flat.reshape(CT, 128, 2, QCOLS).transpose(2, 0, 1, 3))  # [2,CT,128,QC]
        wvTt = np.ascontiguousarray(wv[rows].T.reshape(CT, 128, QCOLS))
        wpTt = np.ascontiguousarray(
            w_proj[:, rows].T.reshape(NP, 128, C)).astype(bf)
        in_maps.append({
            "xT": xTt,
            "wqkT": wqkTt,
            "wvT": wvTt,
            "wpT": wpTt,
            "bias": (np.ascontiguousarray(b_proj.reshape(C // 128, 128).T)
                     if g == 0 else np.zeros((128, C // 128), np.float32)),
        })
    return in_maps


def unshard_output(outT_tiles_pair, TQ=512):
    """outT [NO,NTB,128,TQ] partials (2 cores) -> out [T, C]."""
    s = outT_tiles_pair[0] + outT_tiles_pair[1]
    NO, NTB = C // 128, T // TQ
    return s.transpose(0, 2, 1, 3).reshape(C, T).T


_NC_CACHE = {}


def kernel(x, w_attn, w_proj, b_proj):
    if "nc" not in _NC_CACHE:
        _NC_CACHE["nc"] = build()
    nc = _NC_CACHE["nc"]
    in_maps = shard_inputs(x, w_attn, w_proj, b_proj)
    res = run_bass_kernel_spmd(nc, in_maps, core_ids=list(range(8)))
    out = np.empty((B, T, C), np.float32)
    for b in range(B):
        out[b] = unshard_output([res.results[2 * b]["outT"],
                                 res.results[2 * b + 1]["outT"]])
    return out


# revision 63
# speedup vs baseline: 1.0462x; 1.0462x over previous
"""Trainium2 Bass kernel for nn_AttentionLayer (B=4, T=2048, C=1024, H=16).

Sharding (8 cores): core c = (batch b = c//2, head-group g = c%2).
Data parallel on batch, tensor parallel on heads: each core computes the
qkv projection for its 8 heads, causal flash-attention, and a partial
output projection (row split of w_proj). Host sums the two partials per
batch and re-transposes.

Per-core kernel (Bass/Tile):
  phase A: qkv projection in f32r (TF32-like).  Q^T/K^T produced in
           [head_dim, t] layout, V in [t, head_dim] layout with an
           appended ones column; all stored bf16 in SBUF.
  phase B: causal attention per head-pair.  S^T = K^T.T @ Q^T, the two
           heads of a pair issued to PE row-groups (0,0)/(64,0) so they
           run concurrently on HW; causal mask applied on PE by
           accumulating a constant -BIG triangle into the diagonal
           128-col band (exp then yields exact zeros); exp on ACT (PSUM
           f32 in, bf16 out); O^T = [V|1].T @ P^T accumulated in PSUM,
           row 64 gives softmax denominators.  Each pair's AV batch and
           normalization are deferred one pair and run as priority
           filler during the next pair's S/exp stream; o-banks are
           copied to SBUF immediately so the norm chain (one batched
           multi-pass reciprocal for both heads + selector-matmul
           broadcast + DVE multiplies) stays off the critical path.
  phase C: out^T = w_p^T.T @ y^T in bf16 + bias (bias only on g=0).

  Emission uses a virtual-clock list scheduler: attention chunks are
  emitted in dependency order, and whenever the PE stream would stall
  on ACT (exp) latency, projection / output-projection matmuls are
  spliced in as filler.  PSUM rings are dedicated (st / o / grp) so
  long-lived accumulators never alias short-lived group tiles.

All DRAM tensors are host-pre-tiled so every DMA is one contiguous block.
"""
from collections import deque
from contextlib import ExitStack

import numpy as np

import concourse.bacc as bacc
import concourse.mybir as mybir
import concourse.tile as tile
from concourse.bass_utils import run_bass_kernel_spmd

F32 = mybir.dt.float32
F32R = mybir.dt.float32r
BF16 = mybir.dt.bfloat16
AF = mybir.ActivationFunctionType

B, T, C, H = 4, 2048, 1024, 16
HD = C // H          # 64
NH = H // 2          # heads per core: 8
QCOLS = NH * HD      # 512

# virtual-clock cost model (calibrated against TimelineSim, which matched
# HW within 1% on the S+exp ablation), ns
SEM = 120.0      # semaphore propagation to a dependent engine
PE_DRAIN = 273.0  # PE pipeline drain (173) + sem before a consumer sees PSUM

def _mm(fd):
    return (6.0 + fd) / 2.4 + 5.0

def _act(fd):
    return (fd + 222.0) / 1.2 + 60.0

def _dve(fd, fixed=151.0, acc=1.0):
    return (fixed + fd / acc) / 0.96 + 70.0


def build(T=T, C=C, NH=NH, HD=HD, TQ=512, loop_iters=1, variant="full"):
    assert C % 128 == 0 and T % TQ == 0 and TQ % 128 == 0
    NP = NH // 2              # head pairs
    CT = C // 128             # contraction tiles
    NTB = T // TQ             # time blocks
    TT = T // 128             # tk tiles
    NO = C // 128             # out row tiles
    QC = NH * HD
    scale = 1.0 / (HD ** 0.5)

    nc = bacc.Bacc()
    xT = nc.declare_dram_parameter("xT", [CT, NTB, 128, TQ], F32R, isOutput=False)
    wqkT = nc.declare_dram_parameter("wqkT", [2, CT, 128, QC], F32R, isOutput=False)
    wvT = nc.declare_dram_parameter("wvT", [CT, 128, QC], F32R, isOutput=False)
    wpT = nc.declare_dram_parameter("wpT", [NP, 128, C], BF16, isOutput=False)
    bias = nc.declare_dram_parameter("bias", [128, NO], F32, isOutput=False)
    outT = nc.declare_dram_parameter("outT", [NO, NTB, 128, TQ], F32, isOutput=True)

    with tile.TileContext(nc) as tc, ExitStack() as ctx:
        # long-lived pools first (stack allocator)
        const_pool = ctx.enter_context(tc.tile_pool(name="const", bufs=1))
        wqk_pool = ctx.enter_context(tc.tile_pool(name="wqk", bufs=2 * CT))
        wv_pool = ctx.enter_context(tc.tile_pool(name="wv", bufs=CT))
        wp_pool = ctx.enter_context(tc.tile_pool(name="wp", bufs=NP))
        qt_pool = ctx.enter_context(tc.tile_pool(name="qt", bufs=NP * NTB))
        kt_pool = ctx.enter_context(tc.tile_pool(name="kt", bufs=NP * NTB))
        yt_pool = ctx.enter_context(tc.tile_pool(name="yt", bufs=NP * NTB))
        v_pool = ctx.enter_context(tc.tile_pool(name="v", bufs=TT))
        xs_pool = ctx.enter_context(tc.tile_pool(name="xs", bufs=2 * CT))
        pt_pool = ctx.enter_context(tc.tile_pool(name="pt", bufs=6))
        rc_pool = ctx.enter_context(tc.tile_pool(name="rc", bufs=3))
        osb_pool = ctx.enter_context(tc.tile_pool(name="osb", bufs=3))

        bias_sb = const_pool.tile([128, NO], F32, tag="bias", name="bias_sb")
        nc.sync.dma_start(bias_sb[:], bias[:])
        ones_sb = const_pool.tile([128, NH], BF16, tag="ones", name="ones_sb")
        nc.gpsimd.memset(ones_sb[:], 1.0)
        # causal band mask: mask_a[x,y]=1 iff y>=x, applied to the single
        # 128-wide diagonal band of each S tile
        bf = mybir.dt.np(BF16)
        ii = np.arange(128)[:, None]
        # negtri[k, q] = -BIG where q < k: accumulated onto diagonal S bands
        # via an identity-stationary matmul, so exp yields exact zeros there.
        negtri_np = np.where(np.arange(128)[None, :] < ii, -30000.0, 0.0).astype(bf)
        eye_np = np.eye(128, dtype=np.float32).astype(bf)
        sel_np = np.zeros((65, 64), np.float32)
        sel_np[0, :] = 1.0
        sel_np[64, :] = 1.0
        negtri_dram = nc.inline_tensor(negtri_np, name="negtri")
        eye_dram = nc.inline_tensor(eye_np, name="eye")
        sel_dram = nc.inline_tensor(sel_np, name="sel")
        negtri = const_pool.tile([128, 128], BF16, tag="negtri", name="negtri_sb")
        eye_sb = const_pool.tile([128, 128], BF16, tag="eye", name="eye_sb")
        sel_sb = const_pool.tile([65, 64], F32R, tag="sel", name="sel_sb")
        nc.gpsimd.dma_start(negtri[:], negtri_dram[:])
        nc.gpsimd.dma_start(eye_sb[:], eye_dram[:])
        nc.gpsimd.dma_start(sel_sb[:], sel_dram[:])

        # persistent weights (loaded once, resident in SBUF)
        wp_sb = [wp_pool.tile([128, C], BF16, tag="wp", name="wp") for _ in range(NP)]
        for p in range(NP):
            nc.sync.dma_start(wp_sb[p][:], wpT[p])
        wqk_sb = [[wqk_pool.tile([128, QC], F32R, tag="wqk", name="wqk")
                   for _ in range(CT)] for _ in range(2)]
        for half in range(2):
            for c in range(CT):
                nc.sync.dma_start(wqk_sb[half][c][:], wqkT[half, c])
        wv_sb = [wv_pool.tile([128, QC], F32R, tag="wv", name="wv") for _ in range(CT)]
        for c in range(CT):
            nc.sync.dma_start(wv_sb[c][:], wvT[c])

        qt = {}
        kt = {}
        vt = []
        yt = {}

        def body():
            qt.clear(); kt.clear(); vt.clear(); yt.clear()
            ctx2 = ExitStack()
            # PSUM: st 2x2 banks + o 2x1 + grp 2x1 = 8 banks
            st_pool = ctx2.enter_context(tc.tile_pool(name="st", bufs=2, space="PSUM"))
            o_pool = ctx2.enter_context(tc.tile_pool(name="o", bufs=2, space="PSUM"))
            grp_pool = ctx2.enter_context(tc.tile_pool(name="grp", bufs=2, space="PSUM"))

            class Sched:
                """Emission-time list scheduler with virtual engine clocks.

                Filler generators yield None after emitting a chunk of work,
                or a time value ("blocked until") when they cannot emit yet.
                `prio` (deferred AV/norm batches) pre-empts `bulk`
                (projection / output projection); FIFO within each class.
                """
                def __init__(self):
                    self.vpe = 0.0
                    self.vact = 0.0
                    self.vdve = 0.0
                    self.prio = deque()
                    self.bulk = deque()
                    self.force = False
                    self._blk = None

                def pe(self, ns):
                    self.vpe += ns

                def dve(self, ns, after=None):
                    t0 = max(self.vdve, self.vpe if after is None else after)
                    self.vdve = t0 + ns
                    return self.vdve

                def wait_until(self, t):
                    while not self.force and self.vpe < t:
                        yield t

                def _step_filler(self):
                    while self.prio:  # strict FIFO within prio
                        g = self.prio[0]
                        try:
                            b = next(g)
                        except StopIteration:
                            self.prio.popleft()
                            continue
                        if b is None:
                            return True
                        if self._blk is None or b < self._blk:
                            self._blk = b
                        break  # prio front blocked: fall back to bulk
                    for _ in range(len(self.bulk)):
                        g = self.bulk[0]
                        try:
                            b = next(g)
                        except StopIteration:
                            self.bulk.popleft()
                            continue
                        if b is None:
                            return True
                        if self._blk is None or b < self._blk:
                            self._blk = b
                        self.bulk.rotate(-1)  # let later bulk gens try
                    return False

                def fill_to(self, t):
                    while self.vpe < t:
                        self._blk = None
                        if not self._step_filler():
                            if self._blk is None:
                                break  # no filler work left at all
                            self.vpe = max(self.vpe, min(self._blk, t))

                def drain(self, g):
                    for dq in (self.prio, self.bulk):
                        try:
                            dq.remove(g)
                        except ValueError:
                            pass
                    self.force = True
                    for _ in g:
                        pass
                    self.force = False

                def drain_all(self):
                    for dq in (self.prio, self.bulk):
                        while dq:
                            self.drain(dq[0])

            sch = Sched()

            def gen_proj(tb):
                """Phase A for block tb; yields after ~2 matmuls of work."""
                xs = [xs_pool.tile([128, TQ], F32R, tag="xs", name="xs")
                      for _ in range(CT)]
                for c in range(CT):
                    nc.sync.dma_start(xs[c][:], xT[c, tb])
                yield
                with nc.allow_low_precision(reason="bf16 attention operands"):
                    def qk_chunk(half, jp):
                        ps = grp_pool.tile([128, TQ], F32, tag="grp", name="mm")
                        for c in range(CT):
                            nc.tensor.matmul(
                                ps[:], wqk_sb[half][c][:, 128 * jp:128 * (jp + 1)],
                                xs[c][:], start=(c == 0), stop=(c == CT - 1))
                            sch.pe(_mm(TQ))
                            yield
                        pool = qt_pool if half == 0 else kt_pool
                        dst = pool.tile([128, TQ], BF16, tag="t", name="qk")
                        nc.vector.tensor_copy(dst[:], ps[:])
                        sch.dve(_dve(TQ, 120.0))
                        (qt if half == 0 else kt)[(jp, tb)] = dst
                        yield

                    def v_chunk(ti):
                        ps = grp_pool.tile([128, QC], F32, tag="grp", name="mmv")
                        for c in range(CT):
                            nc.tensor.matmul(ps[:], xs[c][:, 128 * ti:128 * (ti + 1)],
                                             wv_sb[c][:], start=(c == 0), stop=(c == CT - 1))
                            sch.pe(_mm(QC))
                            yield
                        vtile = v_pool.tile([128, NH * (HD + 1)], BF16, tag="v", name="v")
                        v3 = vtile[:].rearrange("p (h d) -> p h d", d=HD + 1)
                        nc.vector.tensor_copy(v3[:, :, 0:HD],
                                              ps[:].rearrange("p (h d) -> p h d", d=HD))
                        nc.vector.tensor_copy(v3[:, :, HD], ones_sb[:])
                        sch.dve(_dve(QC, 120.0) + _dve(NH, 120.0))
                        assert len(vt) == tb * (TQ // 128) + ti
                        vt.append(vtile)
                        yield

                    # pair-0 q/k and all V first so the block's attention can
                    # start while the remaining pairs project as filler
                    yield from qk_chunk(0, 0)
                    yield from qk_chunk(1, 0)
                    for ti in range(TQ // 128):
                        yield from v_chunk(ti)
                    for jp in range(1, NP):
                        yield from qk_chunk(0, jp)
                        yield from qk_chunk(1, jp)

            def gen_outproj(tb):
                """Phase C for block tb (filler work). Must not emit before
                the norm tails that write this block's yt tiles."""
                while norms_done[tb] < NP:
                    yield 1e18
                for ot in range(NO):
                    ps = grp_pool.tile([128, TQ], F32, tag="grp", name="mmo")
                    for p in range(NP):
                        nc.tensor.matmul(ps[:], wp_sb[p][:, 128 * ot:128 * (ot + 1)],
                                         yt[(p, tb)][:], start=(p == 0), stop=(p == NP - 1))
                        sch.pe(_mm(TQ))
                        yield
                    osb = osb_pool.tile([128, TQ], F32, tag="osb", name="osb")
                    nc.vector.tensor_scalar_add(osb[:], ps[:], bias_sb[:, ot:ot + 1])
                    sch.dve(_dve(TQ))
                    nc.sync.dma_start(outT[ot, tb], osb[:])
                    yield

            # deferred-pair state: previous pair awaiting its norm tail
            state = {"prev": None, "o_free": 0.0, "done": 0, "seq": 0, "cur": 0}
            norms_done = [0] * NTB
            pt_const = None
            if variant == "avnodep":
                pt_const = const_pool.tile([128, 2 * TQ], BF16, tag="ptc",
                                           name="ptc", bufs=1)
                nc.gpsimd.memset(pt_const[:], 0.001)

            def gen_deferred(p, qi, avlist, qtile, ntk, myseq):
                """Norm tail of the previous pair, then this pair's AV batch
                (h0 sweep then h1 sweep) and reciprocals. Runs as priority
                filler during the NEXT pair's S/exp stream."""
                with nc.allow_low_precision(reason="bf16 attention operands"):
                    while state["done"] != myseq - 1:
                        yield 1e18  # wait for the previous deferred unit
                    if state["prev"] is not None:
                        emit_normtail()
                        yield None
                    nonorm = variant in ("avnonorm",)
                    # AV batch: o tiles allocated here (ring order = use order).
                    # h0/h1 interleaved per k-tile so each pt slot frees as
                    # early as possible (the next pair's exp stream reuses it)
                    o0 = o_pool.tile([HD + 1, TQ], F32, tag="o", name="o0")
                    o1 = o_pool.tile([HD + 1, TQ], F32, tag="o", name="o1")
                    yield from sch.wait_until(state["o_free"])
                    for tki, (pt, w, dlt) in enumerate(avlist):
                        vtile = vt[tki]
                        v3 = vtile[:].rearrange("p (h d) -> p h d", d=HD + 1)
                        for h, o in ((0, o0), (1, o1)):
                            nc.tensor.matmul(o[:, dlt:TQ], v3[:, 2 * p + h, :],
                                             pt[:, h * w:(h + 1) * w],
                                             start=(tki == 0), stop=(tki == ntk - 1))
                        sch.pe(2 * _mm(w))
                        yield None
                    if not nonorm:
                        # free the PSUM o-banks immediately: copy to SBUF
                        # (and stage the denominator rows into partitions 0
                        # and 64 of one tile); the norm chain then runs
                        # entirely off the copies
                        oc0 = rc_pool.tile([HD + 1, TQ], F32, tag="oc0", name="oc0", bufs=2)
                        oc1 = rc_pool.tile([HD + 1, TQ], F32, tag="oc1", name="oc1", bufs=2)
                        rcf = rc_pool.tile([65, TQ], F32, tag="rcf", name="rcf", bufs=2)
                        nc.vector.tensor_copy(oc0[:], o0[:])
                        nc.vector.tensor_copy(oc1[:], o1[:])
                        nc.vector.tensor_copy(rcf[0:1, :], o0[HD:HD + 1, :])
                        nc.vector.tensor_copy(rcf[64:65, :], o1[HD:HD + 1, :])
                        ofree = sch.dve(4 * _dve(TQ, 120.0), after=sch.vpe + PE_DRAIN)
                        state["o_free"] = ofree + SEM
                        yield None
                        # one reciprocal covers both heads (cost scales with
                        # FD, not partitions): fast Newton-Raphson approx
                        # (~18 bits, 5x faster than multi-pass InstReciprocal)
                        # + f32r rounding copy for the broadcast matmul
                        rc2f = rc_pool.tile([65, TQ], F32, tag="rc2f", name="rc2f", bufs=2)
                        rc2 = rc_pool.tile([65, TQ], F32R, tag="rc2", name="rc2", bufs=2)
                        nc.vector.reciprocal_approx_fast(rc2f[:], rcf[:])
                        nc.vector.tensor_copy(rc2[:], rc2f[:])
                        rcA = rc2[0:1, :]
                        rcB = rc2[64:65, :]
                        sch.dve(3 * _dve(TQ, 58.0))
                        state["prev"] = (p, qi, oc0, oc1, rcA, rcB)
                    state["done"] = myseq

            def emit_normtail():
                p, qi, o0, o1, rcA, rcB = state["prev"]
                bc0 = grp_pool.tile([HD, TQ], F32, tag="grp", name="bc0")
                bc1 = grp_pool.tile([HD, TQ], F32, tag="grp", name="bc1")
                nc.tensor.matmul(bc0[:], sel_sb[0:1, :], rcA[:], start=True, stop=True)
                nc.tensor.matmul(bc1[:], sel_sb[64:65, :], rcB[:], start=True, stop=True)
                sch.pe(2 * _mm(TQ))
                # muls read bc straight from PSUM (other operand is SBUF)
                ytile = yt[(p, qi)]
                nc.vector.tensor_mul(ytile[0:64, :], o0[0:HD, :], bc0[:])
                nc.vector.tensor_mul(ytile[64:128, :], o1[0:HD, :], bc1[:])
                sch.dve(2 * _dve(TQ, 120.0), after=sch.vpe + PE_DRAIN)
                state["prev"] = None
                norms_done[qi] += 1

            def flush_norm():
                """Emit the final pair's norm tail (after its deferred ran)."""
                while state["prev"] is not None or sch.prio:
                    if sch.prio:
                        sch.drain(sch.prio[0])
                    elif state["prev"] is not None:
                        emit_normtail()

            st_free = [0.0, 0.0]

            def ensure_pair(p, qi, ntk):
                """Pull the minimum projection chunks needed for pair (p, qi)
                to be emittable (tiles must exist at emission time)."""
                def ok():
                    return ((p, qi) in qt and len(vt) >= ntk
                            and all((p, tb) in kt for tb in range(qi + 1)))
                for tb in range(qi + 1):
                    g = proj_gens[tb]
                    while not ok():
                        try:
                            next(g)
                        except StopIteration:
                            break
                assert ok()

            def emit_attention(qi):
                state["cur"] = qi
                tq0 = qi * TQ
                ntk = (tq0 + TQ) // 128
                with nc.allow_low_precision(reason="bf16 attention operands"):
                    for p in range(NP):
                        ensure_pair(p, qi, ntk)
                        qtile = qt[(p, qi)]
                        if variant != "sexp":
                            ytile = yt_pool.tile([128, TQ], BF16, tag="yt", name="y")
                            yt[(p, qi)] = ytile
                        avlist = []
                        for tki in range(ntk):
                            tk0 = tki * 128
                            dlt = max(0, tk0 - tq0)
                            w = TQ - dlt
                            diag = tk0 >= tq0
                            ktile = kt[(p, tk0 // TQ)]
                            koff = tk0 % TQ
                            # st ring slot reuse: wait for exp(i-2) to clear
                            sch.fill_to(st_free[tki % 2])
                            st = st_pool.tile([128, 2 * TQ], F32, tag="st", name="st")
                            nc.tensor.matmul(st[:, 0:w], ktile[0:64, koff:koff + 128],
                                             qtile[0:64, dlt:TQ], start=True, stop=not diag)
                            nc.tensor.matmul(st[:, TQ:TQ + w], ktile[64:128, koff:koff + 128],
                                             qtile[64:128, dlt:TQ], start=True, stop=not diag)
                            sch.pe(_mm(w) + 4.0)  # row-groups (0,0)/(64,0) run concurrently
                            if diag:
                                # mask the 128-wide diagonal band: += -BIG triangle
                                nc.tensor.matmul(st[:, 0:128], eye_sb[:], negtri[:],
                                                 start=False, stop=True)
                                nc.tensor.matmul(st[:, TQ:TQ + 128], eye_sb[:], negtri[:],
                                                 start=False, stop=True)
                                sch.pe(2 * _mm(128))
                            pt = pt_pool.tile([128, 2 * w], BF16, tag="pt", name="pt",
                                              padded_shape=[128, 2 * TQ])
                            st_v = st[:].rearrange("p (h q) -> p h q", q=TQ)[:, :, 0:w]
                            pt_v = pt[:].rearrange("p (h q) -> p h q", h=2)
                            nc.scalar.activation(pt_v, st_v, AF.Exp, scale=scale)
                            exp_end = max(sch.vact, sch.vpe + PE_DRAIN) + _act(2 * w)
                            sch.vact = exp_end
                            st_free[tki % 2] = exp_end + SEM
                            avlist.append((pt_const[:, 0:2 * w] if pt_const is not None
                                           else pt, w, dlt))
                        if variant == "sexp":
                            yt[(p, qi)] = qtile
                            norms_done[qi] += 1
                            continue
                        if variant == "avnonorm":
                            yt[(p, qi)] = qtile
                            norms_done[qi] += 1
                        state["seq"] += 1
                        sch.prio.append(gen_deferred(p, qi, avlist, qtile, ntk,
                                                     state["seq"]))

            # ---- main emission ----
            proj_gens = [gen_proj(tb) for tb in range(NTB)]
            if variant == "projout":
                # ablation: pure PE pipeline (no attention chains)
                for tb in range(NTB):
                    sch.drain(proj_gens[tb])
                    for p in range(NP):
                        yt[(p, tb)] = qt[(p, tb)]
                    norms_done[tb] = NP
                    sch.drain(gen_outproj(tb))
                ctx2.close()
                return
            if variant == "serial":
                for tb in range(NTB):
                    sch.drain(proj_gens[tb])
                for qi in range(NTB):
                    emit_attention(qi)
                flush_norm()
                for qi in range(NTB):
                    sch.drain(gen_outproj(qi))
                ctx2.close()
                return
            sch.bulk.append(proj_gens[0])
            for qi in range(NTB):
                if qi + 1 < NTB:
                    sch.bulk.append(proj_gens[qi + 1])
                emit_attention(qi)
                sch.bulk.append(gen_outproj(qi))
            flush_norm()
            sch.drain_all()
            ctx2.close()

        if loop_iters == 1:
            body()
        else:
            with tc.For_i(0, loop_iters, 1):
                body()
    nc.finalize()
    return nc


def _tile2d(a, pr, pc):
    """[R, S] -> [R//pr, S//pc, pr, pc] contiguous tiles."""
    R, S = a.shape
    return np.ascontiguousarray(
        a.reshape(R // pr, pr, S // pc, pc).transpose(0, 2, 1, 3))


def shard_inputs(x, w_attn, w_proj, b_proj, TQ=512):
    """Returns in_maps for 8 cores: core c = (b=c//2, g=c%2)."""
    CT = C // 128
    NP = NH // 2
    bf = mybir.dt.np(BF16)
    wq, wk, wv = w_attn[0:C], w_attn[C:2 * C], w_attn[2 * C:3 * C]
    x = np.asarray(x)
    in_maps = []
    for core in range(8):
        b = core // 2
        g = core % 2
        rows = slice(g * QCOLS, (g + 1) * QCOLS)
        xTt = _tile2d(np.asarray(x[b]).T, 128, TQ)                 # [CT,NTB,128,TQ]
        wqkT_flat = np.concatenate([wq[rows], wk[rows]], 0).T      # [C, 2QC]
        wqkTt = np.ascontiguousarray(
            wqkT_flat.reshape(CT, 128, 2, QCOLS).transpose(2, 0, 1, 3))  # [2,CT,128,QC]
        wvTt = np.ascontiguousarray(wv[rows].T.reshape(CT, 128, QCOLS))
        wpTt = np.ascontiguousarray(
            w_proj[:, rows].T.reshape(NP, 128, C)).astype(bf)
        in_maps.append({
            "xT": xTt,
            "wqkT": wqkTt,
            "wvT": wvTt,
            "wpT": wpTt,
            "bias": (np.ascontiguousarray(b_proj.reshape(C // 128, 128).T)
                     if g == 0 else np.zeros((128, C // 128), np.float32)),
        })
    return in_maps


def unshard_output(outT_tiles_pair, TQ=512):
    """outT [NO,NTB,128,TQ] partials (2 cores) -> out [T, C]."""
    s = outT_tiles_pair[0] + outT_tiles_pair[1]
    NO, NTB = C // 128, T // TQ
    return s.transpose(0, 2, 1, 3).reshape(C, T).T


_NC_CACHE = {}


def kernel(x, w_attn, w_proj, b_proj):
    if "nc" not in _NC_CACHE:
        _NC_CACHE["nc"] = build()
    nc = _NC_CACHE["nc"]
    in_maps = shard_inputs(x, w_attn, w_proj, b_proj)
    res = run_bass_kernel_spmd(nc, in_maps, core_ids=list(range(8)))
    out = np.empty((B, T, C), np.float32)
    for b in range(B):
        out[b] = unshard_output([res.results[2 * b]["outT"],
                                 res.results[2 * b + 1]["outT"]])
    return out


# revision 64
# speedup vs baseline: 1.0465x; 1.0003x over previous
"""Trainium2 Bass kernel for nn_AttentionLayer (B=4, T=2048, C=1024, H=16).

Sharding (8 cores): core c = (batch b = c//2, head-group g = c%2).
Data parallel on batch, tensor parallel on heads: each core computes the
qkv projection for its 8 heads, causal flash-attention, and a partial
output projection (row split of w_proj). Host sums the two partials per
batch and re-transposes.

Per-core kernel (Bass/Tile):
  phase A: qkv projection in f32r (TF32-like).  Q^T/K^T produced in
           [head_dim, t] layout, V in [t, head_dim] layout with an
           appended ones column; all stored bf16 in SBUF.
  phase B: causal attention per head-pair.  S^T = K^T.T @ Q^T, the two
           heads of a pair issued to PE row-groups (0,0)/(64,0) so they
           run concurrently on HW; causal mask applied on PE by
           accumulating a constant -BIG triangle into the diagonal
           128-col band (exp then yields exact zeros); exp on ACT (PSUM
           f32 in, bf16 out); O^T = [V|1].T @ P^T accumulated in PSUM,
           row 64 gives softmax denominators.  Each pair's AV batch and
           normalization are deferred one pair and run as priority
           filler during the next pair's S/exp stream; o-banks are
           copied to SBUF immediately so the norm chain (one batched
           multi-pass reciprocal for both heads + selector-matmul
           broadcast + DVE multiplies) stays off the critical path.
  phase C: out^T = w_p^T.T @ y^T in bf16 + bias (bias only on g=0).

  Emission uses a virtual-clock list scheduler: attention chunks are
  emitted in dependency order, and whenever the PE stream would stall
  on ACT (exp) latency, projection / output-projection matmuls are
  spliced in as filler.  PSUM rings are dedicated (st / o / grp) so
  long-lived accumulators never alias short-lived group tiles.

All DRAM tensors are host-pre-tiled so every DMA is one contiguous block.
"""
from collections import deque
from contextlib import ExitStack

import numpy as np

import concourse.bacc as bacc
import concourse.mybir as mybir
import concourse.tile as tile
from concourse.bass_utils import run_bass_kernel_spmd

F32 = mybir.dt.float32
F32R = mybir.dt.float32r
BF16 = mybir.dt.bfloat16
AF = mybir.ActivationFunctionType

B, T, C, H = 4, 2048, 1024, 16
HD = C // H          # 64
NH = H // 2          # heads per core: 8
QCOLS = NH * HD      # 512

# virtual-clock cost model (calibrated against TimelineSim, which matched
# HW within 1% on the S+exp ablation), ns
SEM = 120.0      # semaphore propagation to a dependent engine
PE_DRAIN = 273.0  # PE pipeline drain (173) + sem before a consumer sees PSUM

def _mm(fd):
    return (6.0 + fd) / 2.4 + 5.0

def _act(fd):
    return (fd + 222.0) / 1.2 + 60.0

def _dve(fd, fixed=151.0, acc=1.0):
    return (fixed + fd / acc) / 0.96 + 70.0


def build(T=T, C=C, NH=NH, HD=HD, TQ=512, loop_iters=1, variant="full"):
    assert C % 128 == 0 and T % TQ == 0 and TQ % 128 == 0
    NP = NH // 2              # head pairs
    CT = C // 128             # contraction tiles
    NTB = T // TQ             # time blocks
    TT = T // 128             # tk tiles
    NO = C // 128             # out row tiles
    QC = NH * HD
    scale = 1.0 / (HD ** 0.5)

    nc = bacc.Bacc()
    xT = nc.declare_dram_parameter("xT", [CT, NTB, 128, TQ], F32R, isOutput=False)
    wqkT = nc.declare_dram_parameter("wqkT", [2, CT, 128, QC], F32R, isOutput=False)
    wvT = nc.declare_dram_parameter("wvT", [CT, 128, QC], F32R, isOutput=False)
    wpT = nc.declare_dram_parameter("wpT", [NP, 128, C], BF16, isOutput=False)
    bias = nc.declare_dram_parameter("bias", [128, NO], F32, isOutput=False)
    outT = nc.declare_dram_parameter("outT", [NO, NTB, 128, TQ], F32, isOutput=True)

    with tile.TileContext(nc) as tc, ExitStack() as ctx:
        # long-lived pools first (stack allocator)
        const_pool = ctx.enter_context(tc.tile_pool(name="const", bufs=1))
        wqk_pool = ctx.enter_context(tc.tile_pool(name="wqk", bufs=2 * CT))
        wv_pool = ctx.enter_context(tc.tile_pool(name="wv", bufs=CT))
        wp_pool = ctx.enter_context(tc.tile_pool(name="wp", bufs=NP))
        qt_pool = ctx.enter_context(tc.tile_pool(name="qt", bufs=NP * NTB))
        kt_pool = ctx.enter_context(tc.tile_pool(name="kt", bufs=NP * NTB))
        yt_pool = ctx.enter_context(tc.tile_pool(name="yt", bufs=NP * NTB))
        v_pool = ctx.enter_context(tc.tile_pool(name="v", bufs=TT))
        xs_pool = ctx.enter_context(tc.tile_pool(name="xs", bufs=2 * CT))
        pt_pool = ctx.enter_context(tc.tile_pool(name="pt", bufs=16))
        rc_pool = ctx.enter_context(tc.tile_pool(name="rc", bufs=3))
        osb_pool = ctx.enter_context(tc.tile_pool(name="osb", bufs=3))

        bias_sb = const_pool.tile([128, NO], F32, tag="bias", name="bias_sb")
        nc.sync.dma_start(bias_sb[:], bias[:])
        ones_sb = const_pool.tile([128, NH], BF16, tag="ones", name="ones_sb")
        nc.gpsimd.memset(ones_sb[:], 1.0)
        # causal band mask: mask_a[x,y]=1 iff y>=x, applied to the single
        # 128-wide diagonal band of each S tile
        bf = mybir.dt.np(BF16)
        ii = np.arange(128)[:, None]
        # negtri[k, q] = -BIG where q < k: accumulated onto diagonal S bands
        # via an identity-stationary matmul, so exp yields exact zeros there.
        negtri_np = np.where(np.arange(128)[None, :] < ii, -30000.0, 0.0).astype(bf)
        eye_np = np.eye(128, dtype=np.float32).astype(bf)
        sel_np = np.zeros((65, 64), np.float32)
        sel_np[0, :] = 1.0
        sel_np[64, :] = 1.0
        negtri_dram = nc.inline_tensor(negtri_np, name="negtri")
        eye_dram = nc.inline_tensor(eye_np, name="eye")
        sel_dram = nc.inline_tensor(sel_np, name="sel")
        negtri = const_pool.tile([128, 128], BF16, tag="negtri", name="negtri_sb")
        eye_sb = const_pool.tile([128, 128], BF16, tag="eye", name="eye_sb")
        sel_sb = const_pool.tile([65, 64], F32R, tag="sel", name="sel_sb")
        nc.gpsimd.dma_start(negtri[:], negtri_dram[:])
        nc.gpsimd.dma_start(eye_sb[:], eye_dram[:])
        nc.gpsimd.dma_start(sel_sb[:], sel_dram[:])

        # persistent weights (loaded once, resident in SBUF)
        wp_sb = [wp_pool.tile([128, C], BF16, tag="wp", name="wp") for _ in range(NP)]
        for p in range(NP):
            nc.sync.dma_start(wp_sb[p][:], wpT[p])
        wqk_sb = [[wqk_pool.tile([128, QC], F32R, tag="wqk", name="wqk")
                   for _ in range(CT)] for _ in range(2)]
        for half in range(2):
            for c in range(CT):
                nc.sync.dma_start(wqk_sb[half][c][:], wqkT[half, c])
        wv_sb = [wv_pool.tile([128, QC], F32R, tag="wv", name="wv") for _ in range(CT)]
        for c in range(CT):
            nc.sync.dma_start(wv_sb[c][:], wvT[c])

        qt = {}
        kt = {}
        vt = []
        yt = {}

        def body():
            qt.clear(); kt.clear(); vt.clear(); yt.clear()
            ctx2 = ExitStack()
            # PSUM: st 2x2 banks + o 2x1 + grp 2x1 = 8 banks
            st_pool = ctx2.enter_context(tc.tile_pool(name="st", bufs=2, space="PSUM"))
            o_pool = ctx2.enter_context(tc.tile_pool(name="o", bufs=2, space="PSUM"))
            grp_pool = ctx2.enter_context(tc.tile_pool(name="grp", bufs=2, space="PSUM"))

            class Sched:
                """Emission-time list scheduler with virtual engine clocks.

                Filler generators yield None after emitting a chunk of work,
                or a time value ("blocked until") when they cannot emit yet.
                `prio` (deferred AV/norm batches) pre-empts `bulk`
                (projection / output projection); FIFO within each class.
                """
                def __init__(self):
                    self.vpe = 0.0
                    self.vact = 0.0
                    self.vdve = 0.0
                    self.prio = deque()
                    self.bulk = deque()
                    self.force = False
                    self._blk = None

                def pe(self, ns):
                    self.vpe += ns

                def dve(self, ns, after=None):
                    t0 = max(self.vdve, self.vpe if after is None else after)
                    self.vdve = t0 + ns
                    return self.vdve

                def wait_until(self, t):
                    while not self.force and self.vpe < t:
                        yield t

                def _step_filler(self):
                    while self.prio:  # strict FIFO within prio
                        g = self.prio[0]
                        try:
                            b = next(g)
                        except StopIteration:
                            self.prio.popleft()
                            continue
                        if b is None:
                            return True
                        if self._blk is None or b < self._blk:
                            self._blk = b
                        break  # prio front blocked: fall back to bulk
                    for _ in range(len(self.bulk)):
                        g = self.bulk[0]
                        try:
                            b = next(g)
                        except StopIteration:
                            self.bulk.popleft()
                            continue
                        if b is None:
                            return True
                        if self._blk is None or b < self._blk:
                            self._blk = b
                        self.bulk.rotate(-1)  # let later bulk gens try
                    return False

                def fill_to(self, t):
                    while self.vpe < t:
                        self._blk = None
                        if not self._step_filler():
                            if self._blk is None:
                                break  # no filler work left at all
                            self.vpe = max(self.vpe, min(self._blk, t))

                def drain(self, g):
                    for dq in (self.prio, self.bulk):
                        try:
                            dq.remove(g)
                        except ValueError:
                            pass
                    self.force = True
                    for _ in g:
                        pass
                    self.force = False

                def drain_all(self):
                    for dq in (self.prio, self.bulk):
                        while dq:
                            self.drain(dq[0])

            sch = Sched()

            def gen_proj(tb):
                """Phase A for block tb; yields after ~2 matmuls of work."""
                xs = [xs_pool.tile([128, TQ], F32R, tag="xs", name="xs")
                      for _ in range(CT)]
                for c in range(CT):
                    nc.sync.dma_start(xs[c][:], xT[c, tb])
                yield
                with nc.allow_low_precision(reason="bf16 attention operands"):
                    def qk_chunk(half, jp):
                        ps = grp_pool.tile([128, TQ], F32, tag="grp", name="mm")
                        for c in range(CT):
                            nc.tensor.matmul(
                                ps[:], wqk_sb[half][c][:, 128 * jp:128 * (jp + 1)],
                                xs[c][:], start=(c == 0), stop=(c == CT - 1))
                            sch.pe(_mm(TQ))
                            yield
                        pool = qt_pool if half == 0 else kt_pool
                        dst = pool.tile([128, TQ], BF16, tag="t", name="qk")
                        nc.vector.tensor_copy(dst[:], ps[:])
                        sch.dve(_dve(TQ, 120.0))
                        (qt if half == 0 else kt)[(jp, tb)] = dst
                        yield

                    def v_chunk(ti):
                        ps = grp_pool.tile([128, QC], F32, tag="grp", name="mmv")
                        for c in range(CT):
                            nc.tensor.matmul(ps[:], xs[c][:, 128 * ti:128 * (ti + 1)],
                                             wv_sb[c][:], start=(c == 0), stop=(c == CT - 1))
                            sch.pe(_mm(QC))
                            yield
                        vtile = v_pool.tile([128, NH * (HD + 1)], BF16, tag="v", name="v")
                        v3 = vtile[:].rearrange("p (h d) -> p h d", d=HD + 1)
                        nc.vector.tensor_copy(v3[:, :, 0:HD],
                                              ps[:].rearrange("p (h d) -> p h d", d=HD))
                        nc.vector.tensor_copy(v3[:, :, HD], ones_sb[:])
                        sch.dve(_dve(QC, 120.0) + _dve(NH, 120.0))
                        assert len(vt) == tb * (TQ // 128) + ti
                        vt.append(vtile)
                        yield

                    # pair-0 q/k and all V first so the block's attention can
                    # start while the remaining pairs project as filler
                    yield from qk_chunk(0, 0)
                    yield from qk_chunk(1, 0)
                    for ti in range(TQ // 128):
                        yield from v_chunk(ti)
                    for jp in range(1, NP):
                        yield from qk_chunk(0, jp)
                        yield from qk_chunk(1, jp)

            def gen_outproj(tb):
                """Phase C for block tb (filler work). Must not emit before
                the norm tails that write this block's yt tiles."""
                while norms_done[tb] < NP:
                    yield 1e18
                for ot in range(NO):
                    ps = grp_pool.tile([128, TQ], F32, tag="grp", name="mmo")
                    for p in range(NP):
                        nc.tensor.matmul(ps[:], wp_sb[p][:, 128 * ot:128 * (ot + 1)],
                                         yt[(p, tb)][:], start=(p == 0), stop=(p == NP - 1))
                        sch.pe(_mm(TQ))
                        yield
                    osb = osb_pool.tile([128, TQ], F32, tag="osb", name="osb")
                    nc.vector.tensor_scalar_add(osb[:], ps[:], bias_sb[:, ot:ot + 1])
                    sch.dve(_dve(TQ))
                    nc.sync.dma_start(outT[ot, tb], osb[:])
                    yield

            # deferred-pair state: previous pair awaiting its norm tail
            state = {"prev": None, "o_free": 0.0, "done": 0, "seq": 0, "cur": 0}
            norms_done = [0] * NTB
            pt_const = None
            if variant == "avnodep":
                pt_const = const_pool.tile([128, 2 * TQ], BF16, tag="ptc",
                                           name="ptc", bufs=1)
                nc.gpsimd.memset(pt_const[:], 0.001)

            def gen_deferred(p, qi, avlist, qtile, ntk, myseq):
                """Norm tail of the previous pair, then this pair's AV batch
                (h0 sweep then h1 sweep) and reciprocals. Runs as priority
                filler during the NEXT pair's S/exp stream."""
                with nc.allow_low_precision(reason="bf16 attention operands"):
                    while state["done"] != myseq - 1:
                        yield 1e18  # wait for the previous deferred unit
                    if state["prev"] is not None:
                        emit_normtail()
                        yield None
                    nonorm = variant in ("avnonorm",)
                    # AV batch: o tiles allocated here (ring order = use order).
                    # h0/h1 interleaved per k-tile so each pt slot frees as
                    # early as possible (the next pair's exp stream reuses it)
                    o0 = o_pool.tile([HD + 1, TQ], F32, tag="o", name="o0")
                    o1 = o_pool.tile([HD + 1, TQ], F32, tag="o", name="o1")
                    yield from sch.wait_until(state["o_free"])
                    for tki, (pt, w, dlt) in enumerate(avlist):
                        vtile = vt[tki]
                        v3 = vtile[:].rearrange("p (h d) -> p h d", d=HD + 1)
                        for h, o in ((0, o0), (1, o1)):
                            nc.tensor.matmul(o[:, dlt:TQ], v3[:, 2 * p + h, :],
                                             pt[:, h * w:(h + 1) * w],
                                             start=(tki == 0), stop=(tki == ntk - 1))
                        sch.pe(2 * _mm(w))
                        yield None
                    if not nonorm:
                        # free the PSUM o-banks immediately: copy to SBUF
                        # (and stage the denominator rows into partitions 0
                        # and 64 of one tile); the norm chain then runs
                        # entirely off the copies
                        oc0 = rc_pool.tile([HD + 1, TQ], F32, tag="oc0", name="oc0", bufs=2)
                        oc1 = rc_pool.tile([HD + 1, TQ], F32, tag="oc1", name="oc1", bufs=2)
                        rcf = rc_pool.tile([65, TQ], F32, tag="rcf", name="rcf", bufs=1)
                        nc.vector.tensor_copy(oc0[:], o0[:])
                        nc.vector.tensor_copy(oc1[:], o1[:])
                        nc.vector.tensor_copy(rcf[0:1, :], o0[HD:HD + 1, :])
                        nc.vector.tensor_copy(rcf[64:65, :], o1[HD:HD + 1, :])
                        ofree = sch.dve(4 * _dve(TQ, 120.0), after=sch.vpe + PE_DRAIN)
                        state["o_free"] = ofree + SEM
                        yield None
                        # one reciprocal covers both heads (cost scales with
                        # FD, not partitions): fast Newton-Raphson approx
                        # (~18 bits, 5x faster than multi-pass InstReciprocal)
                        # + f32r rounding copy for the broadcast matmul
                        rc2f = rc_pool.tile([65, TQ], F32, tag="rc2f", name="rc2f", bufs=1)
                        rc2 = rc_pool.tile([65, TQ], F32R, tag="rc2", name="rc2", bufs=2)
                        nc.vector.reciprocal_approx_fast(rc2f[:], rcf[:])
                        nc.vector.tensor_copy(rc2[:], rc2f[:])
                        rcA = rc2[0:1, :]
                        rcB = rc2[64:65, :]
                        sch.dve(3 * _dve(TQ, 58.0))
                        state["prev"] = (p, qi, oc0, oc1, rcA, rcB)
                    state["done"] = myseq

            def emit_normtail():
                p, qi, o0, o1, rcA, rcB = state["prev"]
                bc0 = grp_pool.tile([HD, TQ], F32, tag="grp", name="bc0")
                bc1 = grp_pool.tile([HD, TQ], F32, tag="grp", name="bc1")
                nc.tensor.matmul(bc0[:], sel_sb[0:1, :], rcA[:], start=True, stop=True)
                nc.tensor.matmul(bc1[:], sel_sb[64:65, :], rcB[:], start=True, stop=True)
                sch.pe(2 * _mm(TQ))
                # muls read bc straight from PSUM (other operand is SBUF)
                ytile = yt[(p, qi)]
                nc.vector.tensor_mul(ytile[0:64, :], o0[0:HD, :], bc0[:])
                nc.vector.tensor_mul(ytile[64:128, :], o1[0:HD, :], bc1[:])
                sch.dve(2 * _dve(TQ, 120.0), after=sch.vpe + PE_DRAIN)
                state["prev"] = None
                norms_done[qi] += 1

            def flush_norm():
                """Emit the final pair's norm tail (after its deferred ran)."""
                while state["prev"] is not None or sch.prio:
                    if sch.prio:
                        sch.drain(sch.prio[0])
                    elif state["prev"] is not None:
                        emit_normtail()

            st_free = [0.0, 0.0]

            def ensure_pair(p, qi, ntk):
                """Pull the minimum projection chunks needed for pair (p, qi)
                to be emittable (tiles must exist at emission time)."""
                def ok():
                    return ((p, qi) in qt and len(vt) >= ntk
                            and all((p, tb) in kt for tb in range(qi + 1)))
                for tb in range(qi + 1):
                    g = proj_gens[tb]
                    while not ok():
                        try:
                            next(g)
                        except StopIteration:
                            break
                assert ok()

            def emit_attention(qi):
                state["cur"] = qi
                tq0 = qi * TQ
                ntk = (tq0 + TQ) // 128
                with nc.allow_low_precision(reason="bf16 attention operands"):
                    for p in range(NP):
                        ensure_pair(p, qi, ntk)
                        qtile = qt[(p, qi)]
                        if variant != "sexp":
                            ytile = yt_pool.tile([128, TQ], BF16, tag="yt", name="y")
                            yt[(p, qi)] = ytile
                        avlist = []
                        for tki in range(ntk):
                            tk0 = tki * 128
                            dlt = max(0, tk0 - tq0)
                            w = TQ - dlt
                            diag = tk0 >= tq0
                            ktile = kt[(p, tk0 // TQ)]
                            koff = tk0 % TQ
                            # st ring slot reuse: wait for exp(i-2) to clear
                            sch.fill_to(st_free[tki % 2])
                            st = st_pool.tile([128, 2 * TQ], F32, tag="st", name="st")
                            nc.tensor.matmul(st[:, 0:w], ktile[0:64, koff:koff + 128],
                                             qtile[0:64, dlt:TQ], start=True, stop=not diag)
                            nc.tensor.matmul(st[:, TQ:TQ + w], ktile[64:128, koff:koff + 128],
                                             qtile[64:128, dlt:TQ], start=True, stop=not diag)
                            sch.pe(_mm(w) + 4.0)  # row-groups (0,0)/(64,0) run concurrently
                            if diag:
                                # mask the 128-wide diagonal band: += -BIG triangle
                                nc.tensor.matmul(st[:, 0:128], eye_sb[:], negtri[:],
                                                 start=False, stop=True)
                                nc.tensor.matmul(st[:, TQ:TQ + 128], eye_sb[:], negtri[:],
                                                 start=False, stop=True)
                                sch.pe(2 * _mm(128))
                            pt = pt_pool.tile([128, 2 * w], BF16, tag="pt", name="pt",
                                              padded_shape=[128, 2 * TQ])
                            st_v = st[:].rearrange("p (h q) -> p h q", q=TQ)[:, :, 0:w]
                            pt_v = pt[:].rearrange("p (h q) -> p h q", h=2)
                            nc.scalar.activation(pt_v, st_v, AF.Exp, scale=scale)
                            exp_end = max(sch.vact, sch.vpe + PE_DRAIN) + _act(2 * w)
                            sch.vact = exp_end
                            st_free[tki % 2] = exp_end + SEM
                            avlist.append((pt_const[:, 0:2 * w] if pt_const is not None
                                           else pt, w, dlt))
                        if variant == "sexp":
                            yt[(p, qi)] = qtile
                            norms_done[qi] += 1
                            continue
                        if variant == "avnonorm":
                            yt[(p, qi)] = qtile
                            norms_done[qi] += 1
                        state["seq"] += 1
                        sch.prio.append(gen_deferred(p, qi, avlist, qtile, ntk,
                                                     state["seq"]))

            # ---- main emission ----
            proj_gens = [gen_proj(tb) for tb in range(NTB)]
            if variant == "projout":
                # ablation: pure PE pipeline (no attention chains)
                for tb in range(NTB):
                    sch.drain(proj_gens[tb])
                    for p in range(NP):
                        yt[(p, tb)] = qt[(p, tb)]
                    norms_done[tb] = NP
                    sch.drain(gen_outproj(tb))
                ctx2.close()
                return
            if variant == "serial":
                for tb in range(NTB):
                    sch.drain(proj_gens[tb])
                for qi in range(NTB):
                    emit_attention(qi)
                flush_norm()
                for qi in range(NTB):
                    sch.drain(gen_outproj(qi))
                ctx2.close()
                return
            sch.bulk.append(proj_gens[0])
            for qi in range(NTB):
                if qi + 1 < NTB:
                    sch.bulk.append(proj_gens[qi + 1])
                emit_attention(qi)
                sch.bulk.append(gen_outproj(qi))
            flush_norm()
            sch.drain_all()
            ctx2.close()

        if loop_iters == 1:
            body()
        else:
            with tc.For_i(0, loop_iters, 1):
                body()
    nc.finalize()
    return nc


def _tile2d(a, pr, pc):
    """[R, S] -> [R//pr, S//pc, pr, pc] contiguous tiles."""
    R, S = a.shape
    return np.ascontiguousarray(
        a.reshape(R // pr, pr, S // pc, pc).transpose(0, 2, 1, 3))


def shard_inputs(x, w_attn, w_proj, b_proj, TQ=512):
    """Returns in_maps for 8 cores: core c = (b=c//2, g=c%2)."""
    CT = C // 128
    NP = NH // 2
    bf = mybir.dt.np(BF16)
    wq, wk, wv = w_attn[0:C], w_attn[C:2 * C], w_attn[2 * C:3 * C]
    x = np.asarray(x)
    in_maps = []
    for core in range(8):
        b = core // 2
        g = core % 2
        rows = slice(g * QCOLS, (g + 1) * QCOLS)
        xTt = _tile2d(np.asarray(x[b]).T, 128, TQ)                 # [CT,NTB,128,TQ]
        wqkT_flat = np.concatenate([wq[rows], wk[rows]], 0).T      # [C, 2QC]
        wqkTt = np.ascontiguousarray(
            wqkT_flat.reshape(CT, 128, 2, QCOLS).transpose(2, 0, 1, 3))  # [2,CT,128,QC]
        wvTt = np.ascontiguousarray(wv[rows].T.reshape(CT, 128, QCOLS))
        wpTt = np.ascontiguousarray(
            w_proj[:, rows].T.reshape(NP, 128, C)).astype(bf)
        in_maps.append({
            "xT": xTt,
            "wqkT": wqkTt,
            "wvT": wvTt,
            "wpT": wpTt,
            "bias": (np.ascontiguousarray(b_proj.reshape(C // 128, 128).T)
                     if g == 0 else np.zeros((128, C // 128), np.float32)),
        })
    return in_maps


def unshard_output(outT_tiles_pair, TQ=512):
    """outT [NO,NTB,128,TQ] partials (2 cores) -> out [T, C]."""
    s = outT_tiles_pair[0] + outT_tiles_pair[1]
    NO, NTB = C // 128, T // TQ
    return s.transpose(0, 2, 1, 3).reshape(C, T).T


_NC_CACHE = {}


def kernel(x, w_attn, w_proj, b_proj):
    if "nc" not in _NC_CACHE:
        _NC_CACHE["nc"] = build()
    nc = _NC_CACHE["nc"]
    in_maps = shard_inputs(x, w_attn, w_proj, b_proj)
    res = run_bass_kernel_spmd(nc, in_maps, core_ids=list(range(8)))
    out = np.empty((B, T, C), np.float32)
    for b in range(B):
        out[b] = unshard_output([res.results[2 * b]["outT"],
                                 res.results[2 * b + 1]["outT"]])
    return out


# revision 70
# speedup vs baseline: 1.0886x; 1.0402x over previous
"""Trainium2 Bass kernel for nn_AttentionLayer (B=4, T=2048, C=1024, H=16).

Sharding (8 cores): core c = (batch b = c//2, head-group g = c%2).
Data parallel on batch, tensor parallel on heads: each core computes the
qkv projection for its 8 heads, causal flash-attention, and a partial
output projection (row split of w_proj). Host sums the two partials per
batch and re-transposes.

Per-core kernel (Bass/Tile):
  phase A: qkv projection in f32r (TF32-like).  Q^T/K^T produced in
           [head_dim, t] layout, V in [t, head_dim] layout with an
           appended ones column; all stored bf16 in SBUF.
  phase B: causal attention per head-pair.  S^T = K^T.T @ Q^T, the two
           heads of a pair issued to PE row-groups (0,0)/(64,0) so they
           run concurrently on HW; causal mask applied on PE by
           accumulating a constant -BIG triangle into the diagonal
           128-col band (exp then yields exact zeros); exp on ACT (PSUM
           f32 in, bf16 out); O^T = [V|1].T @ P^T accumulated in PSUM,
           row 64 gives softmax denominators.  Each pair's AV batch and
           normalization are deferred one pair and run as priority
           filler during the next pair's S/exp stream; o-banks are
           copied to SBUF immediately so the norm chain (one batched
           multi-pass reciprocal for both heads + selector-matmul
           broadcast + DVE multiplies) stays off the critical path.
  phase C: out^T = w_p^T.T @ y^T in bf16 + bias (bias only on g=0).

  Emission uses a virtual-clock list scheduler: attention chunks are
  emitted in dependency order, and whenever the PE stream would stall
  on ACT (exp) latency, projection / output-projection matmuls are
  spliced in as filler.  PSUM rings are dedicated (st / o / grp) so
  long-lived accumulators never alias short-lived group tiles.

All DRAM tensors are host-pre-tiled so every DMA is one contiguous block.
"""
from collections import deque
from contextlib import ExitStack

import numpy as np

import concourse.bacc as bacc
import concourse.mybir as mybir
import concourse.tile as tile
from concourse.bass_utils import run_bass_kernel_spmd

F32 = mybir.dt.float32
F32R = mybir.dt.float32r
BF16 = mybir.dt.bfloat16
AF = mybir.ActivationFunctionType

B, T, C, H = 4, 2048, 1024, 16
HD = C // H          # 64
NH = H // 2          # heads per core: 8
QCOLS = NH * HD      # 512

# virtual-clock cost model (calibrated against TimelineSim, which matched
# HW within 1% on the S+exp ablation), ns
SEM = 120.0      # semaphore propagation to a dependent engine
PE_DRAIN = 273.0  # PE pipeline drain (173) + sem before a consumer sees PSUM

def _mm(fd):
    return (6.0 + fd) / 2.4 + 5.0

def _act(fd):
    return (fd + 222.0) / 1.2 + 60.0

def _dve(fd, fixed=151.0, acc=1.0):
    return (fixed + fd / acc) / 0.96 + 70.0


def build(T=T, C=C, NH=NH, HD=HD, TQ=512, loop_iters=1, variant="full"):
    assert C % 128 == 0 and T % TQ == 0 and TQ % 128 == 0
    NP = NH // 2              # head pairs
    CT = C // 128             # contraction tiles
    NTB = T // TQ             # time blocks
    TT = T // 128             # tk tiles
    NO = C // 128             # out row tiles
    QC = NH * HD
    scale = 1.0 / (HD ** 0.5)

    nc = bacc.Bacc()
    xT = nc.declare_dram_parameter("xT", [CT, NTB, 128, TQ], F32R, isOutput=False)
    wqkT = nc.declare_dram_parameter("wqkT", [2, CT, 128, QC], F32R, isOutput=False)
    wvT = nc.declare_dram_parameter("wvT", [CT, 128, QC], F32R, isOutput=False)
    wpT = nc.declare_dram_parameter("wpT", [NP, 128, C], BF16, isOutput=False)
    bias = nc.declare_dram_parameter("bias", [128, NO], F32, isOutput=False)
    outT = nc.declare_dram_parameter("outT", [NO, NTB, 128, TQ], F32, isOutput=True)

    with tile.TileContext(nc) as tc, ExitStack() as ctx:
        # long-lived pools first (stack allocator)
        const_pool = ctx.enter_context(tc.tile_pool(name="const", bufs=1))
        wqk_pool = ctx.enter_context(tc.tile_pool(name="wqk", bufs=2 * CT))
        wv_pool = ctx.enter_context(tc.tile_pool(name="wv", bufs=CT))
        wp_pool = ctx.enter_context(tc.tile_pool(name="wp", bufs=NP))
        qt_pool = ctx.enter_context(tc.tile_pool(name="qt", bufs=NP * NTB))
        kt_pool = ctx.enter_context(tc.tile_pool(name="kt", bufs=NP * NTB))
        yt_pool = ctx.enter_context(tc.tile_pool(name="yt", bufs=NP * NTB))
        v_pool = ctx.enter_context(tc.tile_pool(name="v", bufs=TT))
        xs_pool = ctx.enter_context(tc.tile_pool(name="xs", bufs=2 * CT))
        pt_pool = ctx.enter_context(tc.tile_pool(name="pt", bufs=16))
        rc_pool = ctx.enter_context(tc.tile_pool(name="rc", bufs=3))
        osb_pool = ctx.enter_context(tc.tile_pool(name="osb", bufs=3))

        bias_sb = const_pool.tile([128, NO], F32, tag="bias", name="bias_sb")
        nc.sync.dma_start(bias_sb[:], bias[:])
        ones_sb = const_pool.tile([128, NH], BF16, tag="ones", name="ones_sb")
        nc.gpsimd.memset(ones_sb[:], 1.0)
        # causal band mask: mask_a[x,y]=1 iff y>=x, applied to the single
        # 128-wide diagonal band of each S tile
        bf = mybir.dt.np(BF16)
        ii = np.arange(128)[:, None]
        # negtri[k, q] = -BIG where q < k: accumulated onto diagonal S bands
        # via an identity-stationary matmul, so exp yields exact zeros there.
        negtri_np = np.where(np.arange(128)[None, :] < ii, -30000.0, 0.0).astype(bf)
        eye_np = np.eye(128, dtype=np.float32).astype(bf)
        sel_np = np.zeros((65, 64), np.float32)
        sel_np[0, :] = 1.0
        sel_np[64, :] = 1.0
        negtri_dram = nc.inline_tensor(negtri_np, name="negtri")
        eye_dram = nc.inline_tensor(eye_np, name="eye")
        sel_dram = nc.inline_tensor(sel_np, name="sel")
        negtri = const_pool.tile([128, 128], BF16, tag="negtri", name="negtri_sb")
        eye_sb = const_pool.tile([128, 128], BF16, tag="eye", name="eye_sb")
        sel_sb = const_pool.tile([65, 64], F32R, tag="sel", name="sel_sb")
        nc.gpsimd.dma_start(negtri[:], negtri_dram[:])
        nc.gpsimd.dma_start(eye_sb[:], eye_dram[:])
        nc.gpsimd.dma_start(sel_sb[:], sel_dram[:])

        # persistent weights (loaded once, resident in SBUF)
        wp_sb = [wp_pool.tile([128, C], BF16, tag="wp", name="wp") for _ in range(NP)]
        for p in range(NP):
            nc.sync.dma_start(wp_sb[p][:], wpT[p])
        wqk_sb = [[wqk_pool.tile([128, QC], F32R, tag="wqk", name="wqk")
                   for _ in range(CT)] for _ in range(2)]
        for half in range(2):
            for c in range(CT):
                nc.sync.dma_start(wqk_sb[half][c][:], wqkT[half, c])
        wv_sb = [wv_pool.tile([128, QC], F32R, tag="wv", name="wv") for _ in range(CT)]
        for c in range(CT):
            nc.sync.dma_start(wv_sb[c][:], wvT[c])

        qt = {}
        kt = {}
        vt = []
        yt = {}

        def body():
            qt.clear(); kt.clear(); vt.clear(); yt.clear()
            ctx2 = ExitStack()
            # PSUM: st 2x2 banks + o 2x1 + grp 2x1 = 8 banks
            st_pool = ctx2.enter_context(tc.tile_pool(name="st", bufs=2, space="PSUM"))
            o_pool = ctx2.enter_context(tc.tile_pool(name="o", bufs=2, space="PSUM"))
            grp_pool = ctx2.enter_context(tc.tile_pool(name="grp", bufs=2, space="PSUM"))

            class Sched:
                """Emission-time list scheduler with virtual engine clocks.

                Filler generators yield None after emitting a chunk of work,
                or a time value ("blocked until") when they cannot emit yet.
                `prio` (deferred AV/norm batches) pre-empts `bulk`
                (projection / output projection); FIFO within each class.
                """
                def __init__(self):
                    self.vpe = 0.0
                    self.vact = 0.0
                    self.vdve = 0.0
                    self.prio = deque()
                    self.bulk = deque()
                    self.force = False
                    self._blk = None

                def pe(self, ns):
                    self.vpe += ns

                def dve(self, ns, after=None):
                    t0 = max(self.vdve, self.vpe if after is None else after)
                    self.vdve = t0 + ns
                    return self.vdve

                def wait_until(self, t):
                    while not self.force and self.vpe < t:
                        yield t

                def _step_filler(self):
                    while self.prio:  # strict FIFO within prio
                        g = self.prio[0]
                        try:
                            b = next(g)
                        except StopIteration:
                            self.prio.popleft()
                            continue
                        if b is None:
                            return True
                        if self._blk is None or b < self._blk:
                            self._blk = b
                        break  # prio front blocked: fall back to bulk
                    for _ in range(len(self.bulk)):
                        g = self.bulk[0]
                        try:
                            b = next(g)
                        except StopIteration:
                            self.bulk.popleft()
                            continue
                        if b is None:
                            return True
                        if self._blk is None or b < self._blk:
                            self._blk = b
                        self.bulk.rotate(-1)  # let later bulk gens try
                    return False

                def fill_to(self, t):
                    while self.vpe < t:
                        self._blk = None
                        if not self._step_filler():
                            if self._blk is None:
                                break  # no filler work left at all
                            self.vpe = max(self.vpe, min(self._blk, t))

                def drain(self, g):
                    for dq in (self.prio, self.bulk):
                        try:
                            dq.remove(g)
                        except ValueError:
                            pass
                    self.force = True
                    for _ in g:
                        pass
                    self.force = False

                def drain_all(self):
                    for dq in (self.prio, self.bulk):
                        while dq:
                            self.drain(dq[0])

            sch = Sched()

            def gen_proj(tb):
                """Phase A for block tb; yields after ~2 matmuls of work."""
                xs = [xs_pool.tile([128, TQ], F32R, tag="xs", name="xs")
                      for _ in range(CT)]
                for c in range(CT):
                    # gpsimd trigger: the SP stream stalls behind the previous
                    # iteration's outT DMAs, which would delay these loads
                    # past the loop boundary
                    nc.gpsimd.dma_start(xs[c][:], xT[c, tb])
                yield
                with nc.allow_low_precision(reason="bf16 attention operands"):
                    def qk_chunk(half, jp):
                        ps = grp_pool.tile([128, TQ], F32, tag="grp", name="mm")
                        for c in range(CT):
                            nc.tensor.matmul(
                                ps[:], wqk_sb[half][c][:, 128 * jp:128 * (jp + 1)],
                                xs[c][:], start=(c == 0), stop=(c == CT - 1))
                            sch.pe(_mm(TQ))
                            yield
                        pool = qt_pool if half == 0 else kt_pool
                        dst = pool.tile([128, TQ], BF16, tag="t", name="qk")
                        nc.vector.tensor_copy(dst[:], ps[:])
                        sch.dve(_dve(TQ, 120.0))
                        (qt if half == 0 else kt)[(jp, tb)] = dst
                        yield

                    def v_chunk(ti):
                        ps = grp_pool.tile([128, QC], F32, tag="grp", name="mmv")
                        for c in range(CT):
                            nc.tensor.matmul(ps[:], xs[c][:, 128 * ti:128 * (ti + 1)],
                                             wv_sb[c][:], start=(c == 0), stop=(c == CT - 1))
                            sch.pe(_mm(QC))
                            yield
                        vtile = v_pool.tile([128, NH * (HD + 1)], BF16, tag="v", name="v")
                        v3 = vtile[:].rearrange("p (h d) -> p h d", d=HD + 1)
                        nc.vector.tensor_copy(v3[:, :, 0:HD],
                                              ps[:].rearrange("p (h d) -> p h d", d=HD))
                        nc.vector.tensor_copy(v3[:, :, HD], ones_sb[:])
                        sch.dve(_dve(QC, 120.0) + _dve(NH, 120.0))
                        assert len(vt) == tb * (TQ // 128) + ti
                        vt.append(vtile)
                        yield

                    # pair-0 q/k and all V first so the block's attention can
                    # start while the remaining pairs project as filler
                    yield from qk_chunk(0, 0)
                    yield from qk_chunk(1, 0)
                    for ti in range(TQ // 128):
                        yield from v_chunk(ti)
                    for jp in range(1, NP):
                        yield from qk_chunk(0, jp)
                        yield from qk_chunk(1, jp)

            def gen_outproj(tb):
                """Phase C for block tb (filler work). Must not emit before
                the norm tails that write this block's yt tiles."""
                while norms_done[tb] < NP:
                    yield 1e18
                for ot in range(NO):
                    ps = grp_pool.tile([128, TQ], F32, tag="grp", name="mmo")
                    for p in range(NP):
                        nc.tensor.matmul(ps[:], wp_sb[p][:, 128 * ot:128 * (ot + 1)],
                                         yt[(p, tb)][:], start=(p == 0), stop=(p == NP - 1))
                        sch.pe(_mm(TQ))
                        yield
                    osb = osb_pool.tile([128, TQ], F32, tag="osb", name="osb")
                    nc.vector.tensor_scalar_add(osb[:], ps[:], bias_sb[:, ot:ot + 1])
                    sch.dve(_dve(TQ))
                    nc.sync.dma_start(outT[ot, tb], osb[:])
                    yield

            # deferred-pair state: previous pair awaiting its norm tail
            state = {"prev": None, "o_free": 0.0, "done": 0, "seq": 0}
            norms_done = [0] * NTB
            pt_const = None
            if variant == "avnodep":
                pt_const = const_pool.tile([128, 2 * TQ], BF16, tag="ptc",
                                           name="ptc", bufs=1)
                nc.gpsimd.memset(pt_const[:], 0.001)

            def gen_deferred(p, qi, avlist, qtile, ntk, myseq):
                """Norm tail of the previous pair, then this pair's AV batch
                (h0 sweep then h1 sweep) and reciprocals. Runs as priority
                filler during the NEXT pair's S/exp stream."""
                with nc.allow_low_precision(reason="bf16 attention operands"):
                    while state["done"] != myseq - 1:
                        yield 1e18  # wait for the previous deferred unit
                    if state["prev"] is not None:
                        emit_normtail()
                        yield None
                    nonorm = variant in ("avnonorm",)
                    # AV batch: o tiles allocated here (ring order = use order).
                    # h0/h1 interleaved per k-tile so each pt slot frees as
                    # early as possible (the next pair's exp stream reuses it)
                    o0 = o_pool.tile([HD + 1, TQ], F32, tag="o", name="o0")
                    o1 = o_pool.tile([HD + 1, TQ], F32, tag="o", name="o1")
                    while len(vt) < ntk:
                        yield 1e18  # V tiles not yet projected; let bulk fill
                    yield from sch.wait_until(state["o_free"])
                    for tki, (pt, w, dlt) in enumerate(avlist):
                        vtile = vt[tki]
                        v3 = vtile[:].rearrange("p (h d) -> p h d", d=HD + 1)
                        for h, o in ((0, o0), (1, o1)):
                            nc.tensor.matmul(o[:, dlt:TQ], v3[:, 2 * p + h, :],
                                             pt[:, h * w:(h + 1) * w],
                                             start=(tki == 0), stop=(tki == ntk - 1))
                        sch.pe(2 * _mm(w))
                        yield None
                    if not nonorm:
                        # free the PSUM o-banks immediately: copy to SBUF
                        # (and stage the denominator rows into partitions 0
                        # and 64 of one tile); the norm chain then runs
                        # entirely off the copies
                        oc0 = rc_pool.tile([HD + 1, TQ], F32, tag="oc0", name="oc0", bufs=2)
                        oc1 = rc_pool.tile([HD + 1, TQ], F32, tag="oc1", name="oc1", bufs=2)
                        rcf = rc_pool.tile([65, TQ], F32, tag="rcf", name="rcf", bufs=1)
                        nc.vector.tensor_copy(oc0[:], o0[:])
                        nc.vector.tensor_copy(oc1[:], o1[:])
                        nc.vector.tensor_copy(rcf[0:1, :], o0[HD:HD + 1, :])
                        nc.vector.tensor_copy(rcf[64:65, :], o1[HD:HD + 1, :])
                        ofree = sch.dve(4 * _dve(TQ, 120.0), after=sch.vpe + PE_DRAIN)
                        state["o_free"] = ofree + SEM
                        yield None
                        # one reciprocal covers both heads (cost scales with
                        # FD, not partitions): fast Newton-Raphson approx
                        # (~18 bits, 5x faster than multi-pass InstReciprocal)
                        # + f32r rounding copy for the broadcast matmul
                        rc2f = rc_pool.tile([65, TQ], F32, tag="rc2f", name="rc2f", bufs=1)
                        rc2 = rc_pool.tile([65, TQ], F32R, tag="rc2", name="rc2", bufs=2)
                        nc.vector.reciprocal_approx_fast(rc2f[:], rcf[:])
                        nc.vector.tensor_copy(rc2[:], rc2f[:])
                        rcA = rc2[0:1, :]
                        rcB = rc2[64:65, :]
                        sch.dve(3 * _dve(TQ, 58.0))
                        state["prev"] = (p, qi, oc0, oc1, rcA, rcB)
                    state["done"] = myseq

            def emit_normtail():
                p, qi, o0, o1, rcA, rcB = state["prev"]
                bc0 = grp_pool.tile([HD, TQ], F32, tag="grp", name="bc0")
                bc1 = grp_pool.tile([HD, TQ], F32, tag="grp", name="bc1")
                nc.tensor.matmul(bc0[:], sel_sb[0:1, :], rcA[:], start=True, stop=True)
                nc.tensor.matmul(bc1[:], sel_sb[64:65, :], rcB[:], start=True, stop=True)
                sch.pe(2 * _mm(TQ))
                # muls read bc straight from PSUM (other operand is SBUF)
                ytile = yt[(p, qi)]
                nc.vector.tensor_mul(ytile[0:64, :], o0[0:HD, :], bc0[:])
                nc.vector.tensor_mul(ytile[64:128, :], o1[0:HD, :], bc1[:])
                sch.dve(2 * _dve(TQ, 120.0), after=sch.vpe + PE_DRAIN)
                state["prev"] = None
                norms_done[qi] += 1

            def flush_norm():
                """Emit the final pair's norm tail (after its deferred ran)."""
                for g in proj_gens:
                    sch.drain(g)  # deferred AV batches block on vt otherwise
                while state["prev"] is not None or sch.prio:
                    if sch.prio:
                        sch.drain(sch.prio[0])
                    elif state["prev"] is not None:
                        emit_normtail()

            st_free = [0.0, 0.0]

            def ensure_pair(p, qi, ntk):
                """Pull the minimum projection chunks needed for pair (p, qi)
                to be emittable (tiles must exist at emission time)."""
                def ok():
                    return ((p, qi) in qt
                            and all((p, tb) in kt for tb in range(qi + 1)))
                for tb in range(qi + 1):
                    g = proj_gens[tb]
                    while not ok():
                        try:
                            next(g)
                        except StopIteration:
                            break
                assert ok()

            def emit_attention(qi):
                tq0 = qi * TQ
                ntk = (tq0 + TQ) // 128
                with nc.allow_low_precision(reason="bf16 attention operands"):
                    for p in range(NP):
                        ensure_pair(p, qi, ntk)
                        qtile = qt[(p, qi)]
                        if variant != "sexp":
                            ytile = yt_pool.tile([128, TQ], BF16, tag="yt", name="y")
                            yt[(p, qi)] = ytile
                        avlist = []
                        for tki in range(ntk):
                            tk0 = tki * 128
                            dlt = max(0, tk0 - tq0)
                            w = TQ - dlt
                            diag = tk0 >= tq0
                            ktile = kt[(p, tk0 // TQ)]
                            koff = tk0 % TQ
                            # st ring slot reuse: wait for exp(i-2) to clear
                            sch.fill_to(st_free[tki % 2])
                            st = st_pool.tile([128, 2 * TQ], F32, tag="st", name="st")
                            nc.tensor.matmul(st[:, 0:w], ktile[0:64, koff:koff + 128],
                                             qtile[0:64, dlt:TQ], start=True, stop=not diag)
                            nc.tensor.matmul(st[:, TQ:TQ + w], ktile[64:128, koff:koff + 128],
                                             qtile[64:128, dlt:TQ], start=True, stop=not diag)
                            sch.pe(_mm(w) + 4.0)  # row-groups (0,0)/(64,0) run concurrently
                            if diag:
                                # mask the 128-wide diagonal band: += -BIG triangle
                                nc.tensor.matmul(st[:, 0:128], eye_sb[:], negtri[:],
                                                 start=False, stop=True)
                                nc.tensor.matmul(st[:, TQ:TQ + 128], eye_sb[:], negtri[:],
                                                 start=False, stop=True)
                                sch.pe(2 * _mm(128))
                            pt = pt_pool.tile([128, 2 * w], BF16, tag="pt", name="pt",
                                              padded_shape=[128, 2 * TQ])
                            st_v = st[:].rearrange("p (h q) -> p h q", q=TQ)[:, :, 0:w]
                            pt_v = pt[:].rearrange("p (h q) -> p h q", h=2)
                            nc.scalar.activation(pt_v, st_v, AF.Exp, scale=scale)
                            exp_end = max(sch.vact, sch.vpe + PE_DRAIN) + _act(2 * w)
                            sch.vact = exp_end
                            st_free[tki % 2] = exp_end + SEM
                            avlist.append((pt_const[:, 0:2 * w] if pt_const is not None
                                           else pt, w, dlt))
                        if variant == "sexp":
                            yt[(p, qi)] = qtile
                            norms_done[qi] += 1
                            continue
                        if variant == "avnonorm":
                            yt[(p, qi)] = qtile
                            norms_done[qi] += 1
                        state["seq"] += 1
                        sch.prio.append(gen_deferred(p, qi, avlist, qtile, ntk,
                                                     state["seq"]))

            # ---- main emission ----
            proj_gens = [gen_proj(tb) for tb in range(NTB)]
            if variant == "projout":
                # ablation: pure PE pipeline (no attention chains)
                for tb in range(NTB):
                    sch.drain(proj_gens[tb])
                    for p in range(NP):
                        yt[(p, tb)] = qt[(p, tb)]
                    norms_done[tb] = NP
                    sch.drain(gen_outproj(tb))
                ctx2.close()
                return
            if variant == "serial":
                for tb in range(NTB):
                    sch.drain(proj_gens[tb])
                for qi in range(NTB):
                    emit_attention(qi)
                flush_norm()
                for qi in range(NTB):
                    sch.drain(gen_outproj(qi))
                ctx2.close()
                return
            sch.bulk.append(proj_gens[0])
            for qi in range(NTB):
                if qi + 1 < NTB:
                    sch.bulk.append(proj_gens[qi + 1])
                emit_attention(qi)
                sch.bulk.append(gen_outproj(qi))
            flush_norm()
            sch.drain_all()
            ctx2.close()

        if loop_iters == 1:
            body()
        else:
            with tc.For_i(0, loop_iters, 1):
                body()
    nc.finalize()
    return nc


def _tile2d(a, pr, pc):
    """[R, S] -> [R//pr, S//pc, pr, pc] contiguous tiles."""
    R, S = a.shape
    return np.ascontiguousarray(
        a.reshape(R // pr, pr, S // pc, pc).transpose(0, 2, 1, 3))


def shard_inputs(x, w_attn, w_proj, b_proj, TQ=512):
    """Returns in_maps for 8 cores: core c = (b=c//2, g=c%2)."""
    CT = C // 128
    NP = NH // 2
    bf = mybir.dt.np(BF16)
    wq, wk, wv = w_attn[0:C], w_attn[C:2 * C], w_attn[2 * C:3 * C]
    x = np.asarray(x)
    in_maps = []
    for core in range(8):
        b = core // 2
        g = core % 2
        rows = slice(g * QCOLS, (g + 1) * QCOLS)
        xTt = _tile2d(np.asarray(x[b]).T, 128, TQ)                 # [CT,NTB,128,TQ]
        wqkT_flat = np.concatenate([wq[rows], wk[rows]], 0).T      # [C, 2QC]
        wqkTt = np.ascontiguousarray(
            wqkT_flat.reshape(CT, 128, 2, QCOLS).transpose(2, 0, 1, 3))  # [2,CT,128,QC]
        wvTt = np.ascontiguousarray(wv[rows].T.reshape(CT, 128, QCOLS))
        wpTt = np.ascontiguousarray(
            w_proj[:, rows].T.reshape(NP, 128, C)).astype(bf)
        in_maps.append({
            "xT": xTt,
            "wqkT": wqkTt,
            "wvT": wvTt,
            "wpT": wpTt,
            "bias": (np.ascontiguousarray(b_proj.reshape(C // 128, 128).T)
                     if g == 0 else np.zeros((128, C // 128), np.float32)),
        })
    return in_maps


def unshard_output(outT_tiles_pair, TQ=512):
    """outT [NO,NTB,128,TQ] partials (2 cores) -> out [T, C]."""
    s = outT_tiles_pair[0] + outT_tiles_pair[1]
    NO, NTB = C // 128, T // TQ
    return s.transpose(0, 2, 1, 3).reshape(C, T).T


_NC_CACHE = {}


def kernel(x, w_attn, w_proj, b_proj):
    if "nc" not in _NC_CACHE:
        _NC_CACHE["nc"] = build()
    nc = _NC_CACHE["nc"]
    in_maps = shard_inputs(x, w_attn, w_proj, b_proj)
    res = run_bass_kernel_spmd(nc, in_maps, core_ids=list(range(8)))
    out = np.empty((B, T, C), np.float32)
    for b in range(B):
        out[b] = unshard_output([res.results[2 * b]["outT"],
                                 res.results[2 * b + 1]["outT"]])
    return out
